# revision 23
# baseline (speedup 1.0000x reference)
"""Trainium2 Bass kernel for nn_CM_NTM_29566554866014 (scatter_memory).

Sharding: pure batch data-parallelism across 8 NeuronCores (B=2048 -> 256/core).
Small parameters replicated. The cross-NTM loop (T=4) is sequential but
batch-local, so each core runs all 4 steps on its batch shard independently.
No collectives.

Key structural facts used (verified against the reference math):
  * The write head (Ww/bw/ww0) and the memory erase/add update are dead code:
    `mem` is reassigned to `mem0[i+1]` each iteration and outputs depend only
    on h and r. They are therefore not computed.
  * Only read0[T-1] is consumed.
  * Per-step state (mem0/h0/c0/wr0) are fresh inputs each step; the only
    sequential dependency across steps is the read vector r.

Layouts:
  * Matmul stack is feature-major ([feat, batch] with feat on partitions) so
    contractions run on the tensor engine with host-pre-transposed weights.
  * NTM addressing is batch-major ([batch, N] / [batch, N, M]) so softmax /
    shift / sharpen are free-dim ops. mem0 is uploaded bf16 (SBUF fit + DVE),
    products accumulate to fp32.
"""

import numpy as np
import ml_dtypes
from contextlib import ExitStack

import concourse.bass as bass
import concourse.tile as tile
from concourse import bacc
from concourse import mybir
from concourse.bass_utils import run_bass_kernel_spmd
from concourse.masks import make_identity

AF = mybir.ActivationFunctionType
ALU = mybir.AluOpType
AX = mybir.AxisListType
FP = mybir.dt.float32
BF = mybir.dt.bfloat16

T, E, V, H, N, M, B = 4, 512, 256, 512, 128, 64, 2048
NCORES = 8
BS = B // NCORES      # 256 batch rows per core
NBT = BS // 128       # 2 batch tiles
HC = H // 128         # 4
EC = E // 128         # 4
VC = V // 128         # 2
ZC = (4 * H) // 128   # 16
NGRP = 2              # n-groups for mem scratch
NGS = N // NGRP       # 16
EPS = 1e-16


def _bcast_inner(ap, count):
    """View `ap` ([P, F]) as [P, F, count] with a stride-0 innermost dim."""
    return bass.AP(tensor=ap.tensor, offset=ap.offset,
                   ap=[*ap.ap, [0, count]])


def _bcast_mid(ap, count):
    """View `ap` ([P, F]) as [P, count, F] with a stride-0 middle dim."""
    return bass.AP(tensor=ap.tensor, offset=ap.offset,
                   ap=[ap.ap[0], [0, count], ap.ap[1]])


def _swap_free(ap):
    """Swap the two free dims of a 3-dim AP ([P, A, B] -> [P, B, A])."""
    return bass.AP(tensor=ap.tensor, offset=ap.offset,
                   ap=[ap.ap[0], ap.ap[2], ap.ap[1]])


def build_nc(stage=None):
    import os
    if stage is None:
        stage = int(os.environ.get("NTM_STAGE", "99"))
    nc = bacc.Bacc()
    d = {}

    def din(name, shape, dt=FP):
        d[name] = nc.dram_tensor(name, list(shape), dt, kind="ExternalInput")

    din("xT",   (T, E, BS))
    din("w1t",  (T, E, H))
    din("w2t",  (T, H, V), BF)
    din("wiht", (T, V + M, 4 * H), BF)
    din("whht", (T, H, 4 * H), BF)
    din("wrt",  (T, H, M + 6), BF)
    din("wot",  (T, H + M, E), BF)
    din("h0t",  (T, H, BS), BF)
    din("c0t",  (T, H, BS))
    din("r0t",  (M, BS), BF)
    din("wr0",  (T, BS, N))
    din("mem0", (T, BS, N, M), BF)
    din("b1c",  (T, 128, HC))
    din("lngc", (T, 128, HC))
    din("lnbc", (T, 128, HC))
    din("b2c",  (T, 128, VC))
    din("bzc",  (T, 128, ZC))
    din("bzch", (T, 128, ZC))
    din("brc",  (T, M + 6, 1))
    din("boc",  (T, 128, EC))
    outT = nc.dram_tensor("outT", [T, E, BS], FP, kind="ExternalOutput")

    with tile.TileContext(nc) as tc, ExitStack() as ctx:
        singles = ctx.enter_context(tc.tile_pool(name="singles", bufs=1))
        wpool = ctx.enter_context(tc.tile_pool(name="wpool", bufs=1))
        spool = ctx.enter_context(tc.tile_pool(name="spool", bufs=1))
        apool = ctx.enter_context(tc.tile_pool(name="apool", bufs=1))
        mpool = ctx.enter_context(tc.tile_pool(name="mpool", bufs=1))
        ppool = ctx.enter_context(tc.tile_pool(name="ppool", bufs=1))
        pmm = ctx.enter_context(tc.tile_pool(name="pmm", bufs=1, space="PSUM"))

        ones_t = singles.tile([128, 128], FP, name="ones_t")
        nc.vector.memset(ones_t, 1.0)
        ident = singles.tile([128, 128], FP, name="ident")
        make_identity(nc, ident)
        eps_ln = singles.tile([128, 1], FP, name="eps_ln")
        nc.vector.memset(eps_ln, 1e-5)

        def mm_ps(shape, name, tag="mm", bufs=4):
            return pmm.tile(shape, FP, name=name, tag=tag, bufs=bufs)

        def transpose_to(dst_ap, src_ap, name):
            """PE-transpose src ([p, f], f<=128) into SBUF dst ([f, p])."""
            p, f = src_ap.shape
            ps = mm_ps([f, p], f"tp_{name}", tag="tp", bufs=2)
            nc.tensor.transpose(ps, src_ap, ident[:p, :p])
            nc.scalar.copy(out=dst_ap, in_=ps)

        def tree_m(dst2d, prod, eng=None, tag="trm"):
            """Sum prod [128, G, M(=64)] over innermost m into dst2d [128, G]
            fp32 via pairwise bf16 adds (DVE 2x mode)."""
            eng = eng or nc.vector
            G = prod.shape[1]
            s1 = ppool.tile([128, G, M // 2], BF, name="trm", tag=tag, bufs=2)
            eng.tensor_add(s1, prod[:, :, 0:M // 2], prod[:, :, M // 2:M])
            w = M // 2
            while w > 2:
                hw = w // 2
                eng.tensor_add(s1[:, :, 0:hw], s1[:, :, 0:hw],
                               s1[:, :, hw:w])
                w = hw
            dst3 = bass.AP(tensor=dst2d.tensor, offset=dst2d.offset,
                           ap=[*dst2d.ap, [1, 1]])
            eng.tensor_add(dst3, s1[:, :, 0:1], s1[:, :, 1:2])

        def tree_n(dst3d, prod):
            """Sum prod [128, G(=64), M] over axis 1 into dst3d [128, 1, M]
            fp32 via pairwise bf16 adds on contiguous halves."""
            G = prod.shape[1]
            s1 = ppool.tile([128, G // 2, M], BF, name="trn", tag="trn", bufs=2)
            nc.vector.tensor_add(s1, prod[:, 0:G // 2, :], prod[:, G // 2:G, :])
            w = G // 2
            while w > 2:
                hw = w // 2
                nc.vector.tensor_add(s1[:, 0:hw, :], s1[:, 0:hw, :],
                                     s1[:, hw:w, :])
                w = hw
            nc.vector.tensor_add(dst3d, s1[:, 0:1, :], s1[:, 1:2, :])

        rT_prev = None
        for t in range(T):
            sfx = f"t{t}"
            # ---------------- loads ----------------
            w1 = [wpool.tile([128, H], FP, name=f"w1_{sfx}_{k}", tag="w1",
                             bufs=4) for k in range(4)]
            for k in range(4):
                nc.sync.dma_start(out=w1[k], in_=d["w1t"][t, k * 128:(k + 1) * 128, :])
            w2 = [wpool.tile([128, V], BF, name=f"w2_{sfx}_{k}", tag="w2",
                             bufs=4) for k in range(4)]
            for k in range(4):
                nc.sync.dma_start(out=w2[k], in_=d["w2t"][t, k * 128:(k + 1) * 128, :])
            wih = []
            for k, ksz in enumerate((128, 128, 64)):
                wt = wpool.tile([ksz, 4 * H], BF, name=f"wih_{sfx}_{k}", tag="wih",
                                bufs=3)
                nc.sync.dma_start(out=wt, in_=d["wiht"][t, k * 128:k * 128 + ksz, :])
                wih.append(wt)
            whh = [wpool.tile([128, 4 * H], BF, name=f"whh_{sfx}_{k}", tag="whh",
                              bufs=4) for k in range(4)]
            for k in range(4):
                nc.sync.dma_start(out=whh[k], in_=d["whht"][t, k * 128:(k + 1) * 128, :])
            wr_ = [wpool.tile([128, M + 6], BF, name=f"wr_{sfx}_{k}", tag="wr",
                              bufs=4) for k in range(4)]
            for k in range(4):
                nc.sync.dma_start(out=wr_[k], in_=d["wrt"][t, k * 128:(k + 1) * 128, :])
            wo = []
            for k, ksz in enumerate((128, 128, 128, 128, 64)):
                wt = wpool.tile([ksz, E], BF, name=f"wo_{sfx}_{k}", tag="wo", bufs=5)
                nc.sync.dma_start(out=wt, in_=d["wot"][t, k * 128:k * 128 + ksz, :])
                wo.append(wt)

            xT = [spool.tile([128, BS], FP, name=f"xT_{sfx}_{k}", tag="xT",
                             bufs=4) for k in range(4)]
            h0 = [spool.tile([128, BS], BF, name=f"h0_{sfx}_{k}", tag="h0",
                             bufs=4) for k in range(4)]
            c0 = [spool.tile([128, BS], FP, name=f"c0_{sfx}_{k}", tag="c0",
                             bufs=4) for k in range(4)]
            for k in range(4):
                nc.sync.dma_start(out=xT[k], in_=d["xT"][t, k * 128:(k + 1) * 128, :])
                nc.sync.dma_start(out=h0[k], in_=d["h0t"][t, k * 128:(k + 1) * 128, :])
                nc.sync.dma_start(out=c0[k], in_=d["c0t"][t, k * 128:(k + 1) * 128, :])

            b1c = spool.tile([128, HC], FP, name=f"b1c_{sfx}", tag="b1c", bufs=2)
            lng = spool.tile([128, HC], FP, name=f"lng_{sfx}", tag="lng", bufs=2)
            lnb = spool.tile([128, HC], FP, name=f"lnb_{sfx}", tag="lnb", bufs=2)
            b2c = spool.tile([128, VC], FP, name=f"b2c_{sfx}", tag="b2c", bufs=2)
            bzc = spool.tile([128, ZC], FP, name=f"bzc_{sfx}", tag="bzc", bufs=2)
            bzch = spool.tile([128, ZC], FP, name=f"bzch_{sfx}", tag="bzch", bufs=2)
            brc = spool.tile([M + 6, 1], FP, name=f"brc_{sfx}", tag="brc", bufs=2)
            boc = spool.tile([128, EC], FP, name=f"boc_{sfx}", tag="boc", bufs=2)
            nc.sync.dma_start(out=b1c, in_=d["b1c"][t])
            nc.sync.dma_start(out=lng, in_=d["lngc"][t])
            nc.sync.dma_start(out=lnb, in_=d["lnbc"][t])
            nc.sync.dma_start(out=b2c, in_=d["b2c"][t])
            nc.sync.dma_start(out=bzc, in_=d["bzc"][t])
            nc.sync.dma_start(out=bzch, in_=d["bzch"][t])
            nc.sync.dma_start(out=brc, in_=d["brc"][t])
            nc.sync.dma_start(out=boc, in_=d["boc"][t])

            mem = []
            w0 = []
            for bt in range(NBT):
                mt = mpool.tile([128, N, M], BF, name=f"mem_{sfx}_{bt}", tag="mem",
                                bufs=3)
                nc.sync.dma_start(out=mt, in_=d["mem0"][t, bt * 128:(bt + 1) * 128])
                mem.append(mt)
                wt = spool.tile([128, N], FP, name=f"w0_{sfx}_{bt}", tag="w0", bufs=4)
                nc.sync.dma_start(out=wt, in_=d["wr0"][t, bt * 128:(bt + 1) * 128, :])
                w0.append(wt)

            if t == 0:
                rT_prev = spool.tile([M, BS], BF, name="r0T", tag="rT", bufs=2)
                nc.sync.dma_start(out=rT_prev, in_=d["r0t"][:, :])

            # ---------------- input projection + LN + p ----------------
            a1 = []
            for hc in range(HC):
                ps = mm_ps([128, BS], f"a1_{sfx}_{hc}")
                for k in range(4):
                    nc.tensor.matmul(ps, w1[k][:, hc * 128:(hc + 1) * 128], xT[k],
                                     start=(k == 0), stop=(k == 3))
                a1s = apool.tile([128, BS], FP, name=f"a1s_{sfx}_{hc}", tag="a1",
                                 bufs=4)
                nc.vector.tensor_scalar(out=a1s, in0=ps,
                                        scalar1=b1c[:, hc:hc + 1], scalar2=None,
                                        op0=ALU.add)
                a1.append(a1s)

            ps_sum = mm_ps([128, BS], f"sums_{sfx}")
            for k in range(4):
                nc.tensor.matmul(ps_sum, ones_t, a1[k], start=(k == 0),
                                 stop=(k == 3))
            ps_sq = mm_ps([128, BS], f"sumsq_{sfx}")
            for k in range(4):
                sq = ppool.tile([128, BS], FP, name=f"sq_{sfx}_{k}", tag="sq",
                                bufs=2)
                nc.scalar.square(sq, a1[k])
                nc.tensor.matmul(ps_sq, ones_t, sq, start=(k == 0), stop=(k == 3))

            mu = apool.tile([128, BS], FP, name=f"mu_{sfx}", tag="mu", bufs=1)
            nc.vector.tensor_scalar(out=mu, in0=ps_sum, scalar1=1.0 / H,
                                    scalar2=None, op0=ALU.mult)
            var = apool.tile([128, BS], FP, name=f"var_{sfx}", tag="var", bufs=1)
            nc.scalar.square(var, mu)
            nc.vector.scalar_tensor_tensor(out=var, in0=ps_sq, scalar=1.0 / H,
                                           in1=var, op0=ALU.mult,
                                           op1=ALU.subtract)
            nc.scalar.activation(out=var, in_=var, func=AF.Ln, bias=eps_ln)
            nc.scalar.activation(out=var, in_=var, func=AF.Exp, scale=-0.5)

            lnt = []
            for hc in range(HC):
                nc.vector.tensor_sub(a1[hc], a1[hc], mu)
                nc.vector.tensor_mul(a1[hc], a1[hc], var)
                lt = apool.tile([128, BS], BF, name=f"lnt_{sfx}_{hc}", tag="lnt",
                                bufs=4)
                nc.scalar.activation(out=lt, in_=a1[hc], func=AF.Relu,
                                     bias=lnb[:, hc:hc + 1],
                                     scale=lng[:, hc:hc + 1])
                lnt.append(lt)

            p = []
            for vc in range(VC):
                ps = mm_ps([128, BS], f"p_{sfx}_{vc}")
                for k in range(4):
                    nc.tensor.matmul(ps, w2[k][:, vc * 128:(vc + 1) * 128], lnt[k],
                                     start=(k == 0), stop=(k == 3))
                pt = apool.tile([128, BS], BF, name=f"p_{sfx}_{vc}", tag="p", bufs=2)
                nc.scalar.activation(out=pt, in_=ps, func=AF.Tanh,
                                     bias=b2c[:, vc:vc + 1])
                p.append(pt)

            # ---------------- mem row norms (chain-independent) ----------------
            sqn = []
            for bt in range(NBT):
                n2 = apool.tile([128, N], FP, name=f"n2_{sfx}_{bt}", tag="n2",
                                bufs=4)
                for g in range(NGRP):
                    prod = ppool.tile([128, NGS, M], BF, name=f"prodn_{sfx}",
                                      tag="prodn", bufs=2)
                    seg = mem[bt][:, g * NGS:(g + 1) * NGS, :]
                    nc.scalar.square(prod, seg)
                    tree_m(n2[:, g * NGS:(g + 1) * NGS], prod, eng=nc.gpsimd,
                           tag="trmn")
                nc.scalar.activation(out=n2, in_=n2, func=AF.Ln)
                nc.scalar.activation(out=n2, in_=n2, func=AF.Exp, scale=0.5)
                sqn.append(n2)

            if stage < 2:
                for vc in range(VC):
                    nc.sync.dma_start(out=outT[t, vc * 128:(vc + 1) * 128, :],
                                      in_=p[vc])
                continue

            # ---------------- LSTM (chain starts: needs rT_prev) ----------------
            h = []
            for hc in range(HC):
                gates = []
                for gi in range(4):
                    oc = gi * 4 + hc
                    osl = slice(oc * 128, (oc + 1) * 128)
                    ps = mm_ps([128, BS], f"z_{sfx}_{oc}")
                    nc.tensor.matmul(ps, wih[0][:, osl], p[0], start=True,
                                     stop=False)
                    nc.tensor.matmul(ps, wih[1][:, osl], p[1], start=False,
                                     stop=False)
                    for k in range(4):
                        nc.tensor.matmul(ps, whh[k][:, osl], h0[k], start=False,
                                         stop=False)
                    nc.tensor.matmul(ps, wih[2][:, osl], rT_prev, start=False,
                                     stop=True)
                    gs = apool.tile([128, BS], FP, name=f"g_{sfx}_{oc}", tag="gt",
                                    bufs=4)
                    nc.scalar.activation(out=gs, in_=ps,
                                         func=(AF.Tanh if gi == 2 else AF.Sigmoid),
                                         bias=bzc[:, oc:oc + 1])
                    gates.append(gs)
                gi_, gf_, gg_, go_ = gates
                t2 = apool.tile([128, BS], FP, name=f"ct2_{sfx}_{hc}", tag="ct",
                                bufs=2)
                nc.vector.tensor_mul(t2, gi_, gg_)
                nc.vector.tensor_mul(gf_, gf_, c0[hc])      # gf_ = f*c0
                nc.vector.tensor_add(t2, t2, gf_)           # t2 = c
                nc.scalar.activation(out=t2, in_=t2, func=AF.Tanh)
                ht = apool.tile([128, BS], BF, name=f"h_{sfx}_{hc}", tag="h", bufs=4)
                nc.vector.tensor_mul(ht, go_, t2)
                h.append(ht)

            if stage < 3:
                for k in range(4):
                    nc.sync.dma_start(out=outT[t, k * 128:(k + 1) * 128, :],
                                      in_=h[k])
                continue

            # ---------------- read head ----------------
            ps_or = mm_ps([M + 6, BS], f"or_{sfx}", tag="or", bufs=2)
            for k in range(4):
                nc.tensor.matmul(ps_or, wr_[k], h[k], start=(k == 0), stop=(k == 3))
            ktan = apool.tile([M, BS], FP, name=f"ktan_{sfx}", tag="ktan", bufs=2)
            nc.scalar.activation(out=ktan, in_=ps_or[:M, :], func=AF.Tanh,
                                 bias=brc[:M, :])
            kh6 = apool.tile([6, BS], FP, name=f"kh6_{sfx}", tag="kh6", bufs=2)
            nc.vector.tensor_scalar(out=kh6, in0=ps_or[M:M + 6, :],
                                    scalar1=brc[M:M + 6, :], scalar2=None,
                                    op0=ALU.add)

            if stage < 41:
                nc.sync.dma_start(out=outT[t, 0:M, :], in_=ktan)
                nc.sync.dma_start(out=outT[t, M:M + 6, :], in_=kh6)
                continue

            rT_next = spool.tile([M, BS], BF, name=f"rT_{sfx}", tag="rT", bufs=2)

            for bt in range(NBT):
                bsl = slice(bt * 128, (bt + 1) * 128)
                kT = apool.tile([128, M], BF, name=f"kT_{sfx}_{bt}", tag="kT",
                                bufs=2)
                transpose_to(kT, ktan[:, bsl], f"k_{sfx}_{bt}")
                khT = apool.tile([128, 6], FP, name=f"khT_{sfx}_{bt}", tag="khT",
                                 bufs=2)
                transpose_to(khT, kh6[:, bsl], f"kh_{sfx}_{bt}")

                def sc(nm):
                    return apool.tile([128, 1], FP, name=f"{nm}_{sfx}_{bt}",
                                      tag="sc1", bufs=16)

                def softplus(dst, src):  # ln(1 + exp(x)); head outputs are small
                    nc.scalar.activation(out=dst, in_=src, func=AF.Exp)
                    nc.vector.tensor_scalar(out=dst, in0=dst, scalar1=1.0,
                                            scalar2=None, op0=ALU.add)
                    nc.scalar.activation(out=dst, in_=dst, func=AF.Ln)

                beta = sc("beta")
                softplus(beta, khT[:, 0:1])
                gint = sc("gint")
                nc.scalar.activation(out=gint, in_=khT[:, 1:2], func=AF.Tanh,
                                     scale=0.5)
                nc.vector.tensor_scalar(out=gint, in0=gint, scalar1=0.5,
                                        scalar2=0.5, op0=ALU.mult, op1=ALU.add)
                if stage < 42:
                    nc.sync.dma_start(
                        out=outT[t, bt * 128:(bt + 1) * 128, 0:1], in_=beta)
                    continue
                smx = sc("smx")
                nc.vector.tensor_reduce(out=smx, in_=khT[:, 2:5], axis=AX.X,
                                        op=ALU.max, negate=True)
                s3 = apool.tile([128, 3], FP, name=f"s3_{sfx}_{bt}", tag="s3",
                                bufs=2)
                nc.scalar.activation(out=s3, in_=khT[:, 2:5], func=AF.Exp,
                                     bias=smx)
                ssum = sc("ssum")
                nc.vector.reduce_sum(out=ssum, in_=s3, axis=AX.X)
                nc.vector.reciprocal(out=ssum, in_=ssum)
                nc.vector.tensor_scalar(out=s3, in0=s3, scalar1=ssum,
                                        scalar2=None, op0=ALU.mult)
                gam = sc("gam")
                softplus(gam, khT[:, 5:6])
                nc.vector.tensor_scalar(out=gam, in0=gam, scalar1=1.0,
                                        scalar2=None, op0=ALU.add)
                if stage < 43:
                    nc.sync.dma_start(
                        out=outT[t, bt * 128:(bt + 1) * 128, 0:3], in_=s3)
                    continue
                kn2 = sc("kn2")
                ksq = apool.tile([128, M], FP, name=f"ksq_{sfx}_{bt}", tag="ksq",
                                 bufs=2)
                nc.vector.tensor_mul(ksq, kT, kT)
                nc.vector.reduce_sum(out=kn2, in_=ksq, axis=AX.X)
                nc.scalar.activation(out=kn2, in_=kn2, func=AF.Ln)
                nc.scalar.activation(out=kn2, in_=kn2, func=AF.Exp, scale=0.5)
                if stage < 44:
                    nc.sync.dma_start(
                        out=outT[t, bt * 128:(bt + 1) * 128, 0:1], in_=kn2)
                    continue

                # cosine similarity numerator, then full addressing
                cn = apool.tile([128, N], FP, name=f"cn_{sfx}_{bt}", tag="cn",
                                bufs=2)
                for g in range(NGRP):
                    prod = ppool.tile([128, NGS, M], BF, name=f"prodc_{sfx}",
                                      tag="prod", bufs=2)
                    nc.vector.tensor_mul(prod,
                                         mem[bt][:, g * NGS:(g + 1) * NGS, :],
                                         _bcast_mid(kT, NGS))
                    tree_m(cn[:, g * NGS:(g + 1) * NGS], prod)
                den = apool.tile([128, N], FP, name=f"den_{sfx}_{bt}", tag="den",
                                 bufs=2)
                nc.vector.tensor_scalar(out=den, in0=sqn[bt], scalar1=kn2,
                                        scalar2=EPS, op0=ALU.mult, op1=ALU.add)
                nc.vector.reciprocal(out=den, in_=den)
                nc.vector.tensor_mul(cn, cn, den)
                if stage < 45:
                    nc.sync.dma_start(
                        out=outT[t, bt * 128:(bt + 1) * 128, 0:N], in_=cn)
                    continue
                # wc = softmax(beta * cos)
                nc.vector.tensor_scalar(out=cn, in0=cn, scalar1=beta,
                                        scalar2=None, op0=ALU.mult)
                mx = sc("mx")
                nc.vector.tensor_reduce(out=mx, in_=cn, axis=AX.X, op=ALU.max,
                                        negate=True)
                nc.scalar.activation(out=cn, in_=cn, func=AF.Exp, bias=mx)
                esum = sc("esum")
                nc.vector.reduce_sum(out=esum, in_=cn, axis=AX.X)
                nc.vector.reciprocal(out=esum, in_=esum)
                nc.vector.tensor_scalar(out=cn, in0=cn, scalar1=esum,
                                        scalar2=None, op0=ALU.mult)
                # wg = g*(wc - wprev) + wprev
                nc.vector.tensor_sub(cn, cn, w0[bt])
                nc.vector.tensor_scalar(out=cn, in0=cn, scalar1=gint,
                                        scalar2=None, op0=ALU.mult)
                nc.vector.tensor_add(cn, cn, w0[bt])
                if stage < 46:
                    nc.sync.dma_start(
                        out=outT[t, bt * 128:(bt + 1) * 128, 0:N], in_=cn)
                    continue
                # ws = s0*roll(wg,+1) + s1*wg + s2*roll(wg,-1)
                wmid = apool.tile([128, N], FP, name=f"wmid_{sfx}_{bt}",
                                  tag="wmid", bufs=2)
                nc.vector.tensor_scalar(out=wmid, in0=cn, scalar1=s3[:, 1:2],
                                        scalar2=None, op0=ALU.mult)
                ws = apool.tile([128, N], FP, name=f"ws_{sfx}_{bt}", tag="ws",
                                bufs=2)
                nc.vector.scalar_tensor_tensor(out=ws[:, 1:N], in0=cn[:, 0:N - 1],
                                               scalar=s3[:, 0:1],
                                               in1=wmid[:, 1:N],
                                               op0=ALU.mult, op1=ALU.add)
                nc.vector.scalar_tensor_tensor(out=ws[:, 0:1], in0=cn[:, N - 1:N],
                                               scalar=s3[:, 0:1],
                                               in1=wmid[:, 0:1],
                                               op0=ALU.mult, op1=ALU.add)
                nc.vector.scalar_tensor_tensor(out=wmid[:, 0:N - 1],
                                               in0=cn[:, 1:N],
                                               scalar=s3[:, 2:3],
                                               in1=ws[:, 0:N - 1],
                                               op0=ALU.mult, op1=ALU.add)
                nc.vector.scalar_tensor_tensor(out=wmid[:, N - 1:N],
                                               in0=cn[:, 0:1],
                                               scalar=s3[:, 2:3],
                                               in1=ws[:, N - 1:N],
                                               op0=ALU.mult, op1=ALU.add)
                if stage < 47:
                    nc.sync.dma_start(
                        out=outT[t, bt * 128:(bt + 1) * 128, 0:N], in_=wmid)
                    continue
                # sharpen: w = ws**gamma / (sum + eps)
                nc.scalar.activation(out=wmid, in_=wmid, func=AF.Ln)
                nc.vector.tensor_scalar(out=wmid, in0=wmid, scalar1=gam,
                                        scalar2=None, op0=ALU.mult)
                nc.scalar.activation(out=wmid, in_=wmid, func=AF.Exp)
                wsum = sc("wsum")
                nc.vector.reduce_sum(out=wsum, in_=wmid, axis=AX.X)
                nc.vector.tensor_scalar(out=wsum, in0=wsum, scalar1=EPS,
                                        scalar2=None, op0=ALU.add)
                nc.vector.reciprocal(out=wsum, in_=wsum)
                nc.vector.tensor_scalar(out=wmid, in0=wmid, scalar1=wsum,
                                        scalar2=None, op0=ALU.mult)
                wrb = apool.tile([128, N], BF, name=f"wrb_{sfx}_{bt}", tag="wrb",
                                 bufs=2)
                nc.scalar.copy(out=wrb, in_=wmid)

                if stage < 50:
                    nc.sync.dma_start(
                        out=outT[t, bt * 128:(bt + 1) * 128, 0:N], in_=wmid)
                    continue

                # r = sum_n w[b,n] * mem[b,n,:]
                rp = apool.tile([128, NGRP, M], FP, name=f"rp_{sfx}_{bt}",
                                tag="rp", bufs=1)
                for g in range(NGRP):
                    prod = ppool.tile([128, NGS, M], BF, name=f"prodr_{sfx}",
                                      tag="prod", bufs=2)
                    wseg = wrb[:, g * NGS:(g + 1) * NGS]
                    nc.vector.tensor_mul(prod,
                                         mem[bt][:, g * NGS:(g + 1) * NGS, :],
                                         _bcast_inner(wseg, M))
                    tree_n(rp[:, g:g + 1, :], prod)
                st = 1
                while st < NGRP:
                    for g0 in range(0, NGRP, 2 * st):
                        nc.vector.tensor_add(rp[:, g0, :], rp[:, g0, :],
                                             rp[:, g0 + st, :])
                    st *= 2
                transpose_to(rT_next[:, bsl], rp[:, 0, :], f"r_{sfx}_{bt}")

            if stage < 41:
                continue
            if stage < 99:
                if stage >= 50:
                    nc.sync.dma_start(out=outT[t, 0:M, :], in_=rT_next)
                rT_prev = rT_next if stage >= 50 else rT_prev
                continue

            # ---------------- output projection ----------------
            for ec in range(EC):
                esl = slice(ec * 128, (ec + 1) * 128)
                ps = mm_ps([128, BS], f"o_{sfx}_{ec}")
                for k in range(4):
                    nc.tensor.matmul(ps, wo[k][:, esl], h[k], start=(k == 0),
                                     stop=False)
                nc.tensor.matmul(ps, wo[4][:, esl], rT_next, start=False,
                                 stop=True)
                os_ = apool.tile([128, BS], FP, name=f"os_{sfx}_{ec}", tag="os",
                                 bufs=2)
                nc.scalar.activation(out=os_, in_=ps, func=AF.Tanh, scale=0.5,
                                     bias=boc[:, ec:ec + 1])
                nc.vector.tensor_scalar(out=os_, in0=os_, scalar1=0.5,
                                        scalar2=0.5, op0=ALU.mult, op1=ALU.add)
                nc.sync.dma_start(out=outT[t, esl, :], in_=os_)

            rT_prev = rT_next

    nc.compile()
    return nc


_CACHE = {}
LAST = {}


def _get_nc():
    if "nc" not in _CACHE:
        _CACHE["nc"] = build_nc()
    return _CACHE["nc"]


def host_prep(inputs, W1, b1, lng, lnb, W2, b2, Wih, Whh, bih, bhh,
              Wr, br, Ww, bw, Wo, bo, mem0, read0, wr0, ww0, h0, c0):
    f32 = np.float32
    inputs, W1, W2, Wih, Whh, Wr, Wo = [np.asarray(a, f32) for a in
                                        (inputs, W1, W2, Wih, Whh, Wr, Wo)]

    def percol(v, cols):   # [T, 128*cols] -> [T, 128, cols] column-major chunks
        return np.ascontiguousarray(
            np.asarray(v, f32).reshape(T, cols, 128).transpose(0, 2, 1))

    bf = ml_dtypes.bfloat16
    xT_full = np.ascontiguousarray(inputs.transpose(0, 2, 1))      # [T, E, B]
    w1t = np.ascontiguousarray(W1.transpose(0, 2, 1))              # [T, E, H]
    w2t = np.ascontiguousarray(W2.transpose(0, 2, 1)).astype(bf)   # [T, H, V]
    wiht = np.ascontiguousarray(Wih.transpose(0, 2, 1)).astype(bf)
    whht = np.ascontiguousarray(Whh.transpose(0, 2, 1)).astype(bf)
    wrt = np.ascontiguousarray(Wr.transpose(0, 2, 1)).astype(bf)   # [T, H, 70]
    wot = np.ascontiguousarray(Wo.transpose(0, 2, 1)).astype(bf)   # [T, 576, E]
    h0t_full = np.asarray(h0, f32).transpose(0, 2, 1).astype(bf)
    c0t_full = np.ascontiguousarray(np.asarray(c0, f32).transpose(0, 2, 1))
    r0t_full = np.asarray(read0, f32)[T - 1].T.astype(bf)          # [M, B]
    wr0_full = np.asarray(wr0, f32)
    mem0_full = np.asarray(mem0).astype(ml_dtypes.bfloat16)
    bz = np.asarray(bih, f32) + np.asarray(bhh, f32)

    common = dict(
        w1t=w1t, w2t=w2t, wiht=wiht, whht=whht, wrt=wrt, wot=wot,
        b1c=percol(b1, HC), lngc=percol(lng, HC), lnbc=percol(lnb, HC),
        b2c=percol(b2, VC), bzc=percol(bz, ZC), bzch=percol(0.5 * bz, ZC),
        brc=np.ascontiguousarray(np.asarray(br, f32).reshape(T, M + 6, 1)),
        boc=percol(bo, EC),
    )
    in_maps = []
    for ci in range(NCORES):
        bsl = slice(ci * BS, (ci + 1) * BS)
        in_maps.append(dict(
            common,
            xT=np.ascontiguousarray(xT_full[:, :, bsl]),
            h0t=np.ascontiguousarray(h0t_full[:, :, bsl]),
            c0t=np.ascontiguousarray(c0t_full[:, :, bsl]),
            r0t=np.ascontiguousarray(r0t_full[:, bsl]),
            wr0=np.ascontiguousarray(wr0_full[:, bsl, :]),
            mem0=np.ascontiguousarray(mem0_full[:, bsl]),
        ))

    return in_maps


def kernel(**inputs):
    in_maps = host_prep(**inputs)
    nc = _get_nc()
    import os
    trace = os.environ.get("BASS_TRACE", "") not in ("", "0")
    res = run_bass_kernel_spmd(nc, in_maps, list(range(NCORES)), trace=trace)
    LAST["exec_time_ns"] = res.exec_time_ns
    LAST["results"] = res
    out = np.concatenate(
        [np.transpose(r["outT"], (0, 2, 1)) for r in res.results], axis=1)
    return np.ascontiguousarray(out.astype(np.float32))


# revision 24
# speedup vs baseline: 1.1511x; 1.1511x over previous
"""Trainium2 Bass kernel for nn_CM_NTM_29566554866014 (scatter_memory).

Sharding: pure batch data-parallelism across 8 NeuronCores (B=2048 -> 256/core).
Small parameters replicated. The cross-NTM loop (T=4) is sequential but
batch-local, so each core runs all 4 steps on its batch shard independently.
No collectives.

Key structural facts used (verified against the reference math):
  * The write head (Ww/bw/ww0) and the memory erase/add update are dead code:
    `mem` is reassigned to `mem0[i+1]` each iteration and outputs depend only
    on h and r. They are therefore not computed.
  * Only read0[T-1] is consumed.
  * Per-step state (mem0/h0/c0/wr0) are fresh inputs each step; the only
    sequential dependency across steps is the read vector r.

Layouts:
  * Matmul stack is feature-major ([feat, batch] with feat on partitions) so
    contractions run on the tensor engine with host-pre-transposed weights.
  * NTM addressing is batch-major ([batch, N] / [batch, N, M]) so softmax /
    shift / sharpen are free-dim ops. mem0 is uploaded bf16 (SBUF fit + DVE),
    products accumulate to fp32.
"""

import numpy as np
import ml_dtypes
from contextlib import ExitStack

import concourse.bass as bass
import concourse.tile as tile
from concourse import bacc
from concourse import mybir
from concourse.bass_utils import run_bass_kernel_spmd
from concourse.masks import make_identity

AF = mybir.ActivationFunctionType
ALU = mybir.AluOpType
AX = mybir.AxisListType
FP = mybir.dt.float32
BF = mybir.dt.bfloat16

T, E, V, H, N, M, B = 4, 512, 256, 512, 128, 64, 2048
NCORES = 8
BS = B // NCORES      # 256 batch rows per core
NBT = BS // 128       # 2 batch tiles
HC = H // 128         # 4
EC = E // 128         # 4
VC = V // 128         # 2
ZC = (4 * H) // 128   # 16
NGRP = 2              # n-groups for mem scratch
NGS = N // NGRP       # 16
EPS = 1e-16


def _bcast_inner(ap, count):
    """View `ap` ([P, F]) as [P, F, count] with a stride-0 innermost dim."""
    return bass.AP(tensor=ap.tensor, offset=ap.offset,
                   ap=[*ap.ap, [0, count]])


def _bcast_mid(ap, count):
    """View `ap` ([P, F]) as [P, count, F] with a stride-0 middle dim."""
    return bass.AP(tensor=ap.tensor, offset=ap.offset,
                   ap=[ap.ap[0], [0, count], ap.ap[1]])


def _swap_free(ap):
    """Swap the two free dims of a 3-dim AP ([P, A, B] -> [P, B, A])."""
    return bass.AP(tensor=ap.tensor, offset=ap.offset,
                   ap=[ap.ap[0], ap.ap[2], ap.ap[1]])


def build_nc(stage=None):
    import os
    if stage is None:
        stage = int(os.environ.get("NTM_STAGE", "99"))
    nc = bacc.Bacc()
    d = {}

    def din(name, shape, dt=FP):
        d[name] = nc.dram_tensor(name, list(shape), dt, kind="ExternalInput")

    din("xT",   (T, E, BS))
    din("w1t",  (T, E, H))
    din("w2t",  (T, H, V), BF)
    din("wiht", (T, V + M, 4 * H), BF)
    din("whht", (T, H, 4 * H), BF)
    din("wrt",  (T, H, M + 6), BF)
    din("wot",  (T, H + M, E), BF)
    din("h0t",  (T, H, BS), BF)
    din("c0t",  (T, H, BS))
    din("r0t",  (M, BS), BF)
    din("wr0",  (T, BS, N))
    din("mem0", (T, BS, N, M), BF)
    din("b1c",  (T, 128, HC))
    din("lngc", (T, 128, HC))
    din("lnbc", (T, 128, HC))
    din("b2c",  (T, 128, VC))
    din("bzc",  (T, 128, ZC))
    din("bzch", (T, 128, ZC))
    din("brc",  (T, M + 6, 1))
    din("boc",  (T, 128, EC))
    outT = nc.dram_tensor("outT", [T, E, BS], FP, kind="ExternalOutput")

    with tile.TileContext(nc) as tc, ExitStack() as ctx:
        singles = ctx.enter_context(tc.tile_pool(name="singles", bufs=1))
        wpool = ctx.enter_context(tc.tile_pool(name="wpool", bufs=1))
        spool = ctx.enter_context(tc.tile_pool(name="spool", bufs=1))
        apool = ctx.enter_context(tc.tile_pool(name="apool", bufs=1))
        mpool = ctx.enter_context(tc.tile_pool(name="mpool", bufs=1))
        ppool = ctx.enter_context(tc.tile_pool(name="ppool", bufs=1))
        pmm = ctx.enter_context(tc.tile_pool(name="pmm", bufs=1, space="PSUM"))

        ones_t = singles.tile([128, 128], FP, name="ones_t")
        nc.vector.memset(ones_t, 1.0)
        ident = singles.tile([128, 128], FP, name="ident")
        make_identity(nc, ident)
        eps_ln = singles.tile([128, 1], FP, name="eps_ln")
        nc.vector.memset(eps_ln, 1e-5)

        def mm_ps(shape, name, tag="mm", bufs=4):
            return pmm.tile(shape, FP, name=name, tag=tag, bufs=bufs)

        def transpose_to(dst_ap, src_ap, name):
            """PE-transpose src ([p, f], f<=128) into SBUF dst ([f, p])."""
            p, f = src_ap.shape
            ps = mm_ps([f, p], f"tp_{name}", tag="tp", bufs=2)
            nc.tensor.transpose(ps, src_ap, ident[:p, :p])
            nc.scalar.copy(out=dst_ap, in_=ps)

        def tree_m(dst2d, prod, eng=None, tag="trm"):
            """Sum prod [128, G, M(=64)] over innermost m into dst2d [128, G]
            fp32 via pairwise bf16 adds (DVE 2x mode)."""
            eng = eng or nc.vector
            G = prod.shape[1]
            s1 = ppool.tile([128, G, M // 2], BF, name="trm", tag=tag, bufs=3)
            eng.tensor_add(s1, prod[:, :, 0:M // 2], prod[:, :, M // 2:M])
            w = M // 2
            while w > 2:
                hw = w // 2
                eng.tensor_add(s1[:, :, 0:hw], s1[:, :, 0:hw],
                               s1[:, :, hw:w])
                w = hw
            dst3 = bass.AP(tensor=dst2d.tensor, offset=dst2d.offset,
                           ap=[*dst2d.ap, [1, 1]])
            eng.tensor_add(dst3, s1[:, :, 0:1], s1[:, :, 1:2])

        def tree_n(dst3d, prod):
            """Sum prod [128, G(=64), M] over axis 1 into dst3d [128, 1, M]
            fp32 via pairwise bf16 adds on contiguous halves."""
            G = prod.shape[1]
            s1 = ppool.tile([128, G // 2, M], BF, name="trn", tag="trn", bufs=3)
            nc.vector.tensor_add(s1, prod[:, 0:G // 2, :], prod[:, G // 2:G, :])
            w = G // 2
            while w > 2:
                hw = w // 2
                nc.vector.tensor_add(s1[:, 0:hw, :], s1[:, 0:hw, :],
                                     s1[:, hw:w, :])
                w = hw
            nc.vector.tensor_add(dst3d, s1[:, 0:1, :], s1[:, 1:2, :])

        rT_prev = None
        for t in range(T):
            sfx = f"t{t}"
            # ---------------- loads ----------------
            w1 = [wpool.tile([128, H], FP, name=f"w1_{sfx}_{k}", tag="w1",
                             bufs=4) for k in range(4)]
            for k in range(4):
                nc.sync.dma_start(out=w1[k], in_=d["w1t"][t, k * 128:(k + 1) * 128, :])
            w2 = [wpool.tile([128, V], BF, name=f"w2_{sfx}_{k}", tag="w2",
                             bufs=4) for k in range(4)]
            for k in range(4):
                nc.sync.dma_start(out=w2[k], in_=d["w2t"][t, k * 128:(k + 1) * 128, :])
            wih = []
            for k, ksz in enumerate((128, 128, 64)):
                wt = wpool.tile([ksz, 4 * H], BF, name=f"wih_{sfx}_{k}", tag="wih",
                                bufs=3)
                nc.sync.dma_start(out=wt, in_=d["wiht"][t, k * 128:k * 128 + ksz, :])
                wih.append(wt)
            whh = [wpool.tile([128, 4 * H], BF, name=f"whh_{sfx}_{k}", tag="whh",
                              bufs=4) for k in range(4)]
            for k in range(4):
                nc.sync.dma_start(out=whh[k], in_=d["whht"][t, k * 128:(k + 1) * 128, :])
            wr_ = [wpool.tile([128, M + 6], BF, name=f"wr_{sfx}_{k}", tag="wr",
                              bufs=4) for k in range(4)]
            for k in range(4):
                nc.sync.dma_start(out=wr_[k], in_=d["wrt"][t, k * 128:(k + 1) * 128, :])
            wo = []
            for k, ksz in enumerate((128, 128, 128, 128, 64)):
                wt = wpool.tile([ksz, E], BF, name=f"wo_{sfx}_{k}", tag="wo", bufs=5)
                nc.sync.dma_start(out=wt, in_=d["wot"][t, k * 128:k * 128 + ksz, :])
                wo.append(wt)

            xT = [spool.tile([128, BS], FP, name=f"xT_{sfx}_{k}", tag="xT",
                             bufs=4) for k in range(4)]
            h0 = [spool.tile([128, BS], BF, name=f"h0_{sfx}_{k}", tag="h0",
                             bufs=4) for k in range(4)]
            c0 = [spool.tile([128, BS], FP, name=f"c0_{sfx}_{k}", tag="c0",
                             bufs=4) for k in range(4)]
            for k in range(4):
                nc.sync.dma_start(out=xT[k], in_=d["xT"][t, k * 128:(k + 1) * 128, :])
                nc.sync.dma_start(out=h0[k], in_=d["h0t"][t, k * 128:(k + 1) * 128, :])
                nc.sync.dma_start(out=c0[k], in_=d["c0t"][t, k * 128:(k + 1) * 128, :])

            b1c = spool.tile([128, HC], FP, name=f"b1c_{sfx}", tag="b1c", bufs=2)
            lng = spool.tile([128, HC], FP, name=f"lng_{sfx}", tag="lng", bufs=2)
            lnb = spool.tile([128, HC], FP, name=f"lnb_{sfx}", tag="lnb", bufs=2)
            b2c = spool.tile([128, VC], FP, name=f"b2c_{sfx}", tag="b2c", bufs=2)
            bzc = spool.tile([128, ZC], FP, name=f"bzc_{sfx}", tag="bzc", bufs=2)
            bzch = spool.tile([128, ZC], FP, name=f"bzch_{sfx}", tag="bzch", bufs=2)
            brc = spool.tile([M + 6, 1], FP, name=f"brc_{sfx}", tag="brc", bufs=2)
            boc = spool.tile([128, EC], FP, name=f"boc_{sfx}", tag="boc", bufs=2)
            nc.sync.dma_start(out=b1c, in_=d["b1c"][t])
            nc.sync.dma_start(out=lng, in_=d["lngc"][t])
            nc.sync.dma_start(out=lnb, in_=d["lnbc"][t])
            nc.sync.dma_start(out=b2c, in_=d["b2c"][t])
            nc.sync.dma_start(out=bzc, in_=d["bzc"][t])
            nc.sync.dma_start(out=bzch, in_=d["bzch"][t])
            nc.sync.dma_start(out=brc, in_=d["brc"][t])
            nc.sync.dma_start(out=boc, in_=d["boc"][t])

            mem = []
            w0 = []
            for bt in range(NBT):
                mt = mpool.tile([128, N, M], BF, name=f"mem_{sfx}_{bt}", tag="mem",
                                bufs=3)
                nc.sync.dma_start(out=mt, in_=d["mem0"][t, bt * 128:(bt + 1) * 128])
                mem.append(mt)
                wt = spool.tile([128, N], FP, name=f"w0_{sfx}_{bt}", tag="w0", bufs=4)
                nc.sync.dma_start(out=wt, in_=d["wr0"][t, bt * 128:(bt + 1) * 128, :])
                w0.append(wt)

            if t == 0:
                rT_prev = spool.tile([M, BS], BF, name="r0T", tag="rT", bufs=2)
                nc.sync.dma_start(out=rT_prev, in_=d["r0t"][:, :])

            # ---------------- input projection + LN + p ----------------
            a1 = []
            for hc in range(HC):
                ps = mm_ps([128, BS], f"a1_{sfx}_{hc}")
                for k in range(4):
                    nc.tensor.matmul(ps, w1[k][:, hc * 128:(hc + 1) * 128], xT[k],
                                     start=(k == 0), stop=(k == 3))
                a1s = apool.tile([128, BS], FP, name=f"a1s_{sfx}_{hc}", tag="a1",
                                 bufs=4)
                nc.vector.tensor_scalar(out=a1s, in0=ps,
                                        scalar1=b1c[:, hc:hc + 1], scalar2=None,
                                        op0=ALU.add)
                a1.append(a1s)

            ps_sum = mm_ps([128, BS], f"sums_{sfx}")
            for k in range(4):
                nc.tensor.matmul(ps_sum, ones_t, a1[k], start=(k == 0),
                                 stop=(k == 3))
            ps_sq = mm_ps([128, BS], f"sumsq_{sfx}")
            for k in range(4):
                sq = ppool.tile([128, BS], FP, name=f"sq_{sfx}_{k}", tag="sq",
                                bufs=2)
                nc.scalar.square(sq, a1[k])
                nc.tensor.matmul(ps_sq, ones_t, sq, start=(k == 0), stop=(k == 3))

            mu = apool.tile([128, BS], FP, name=f"mu_{sfx}", tag="mu", bufs=1)
            nc.vector.tensor_scalar(out=mu, in0=ps_sum, scalar1=1.0 / H,
                                    scalar2=None, op0=ALU.mult)
            var = apool.tile([128, BS], FP, name=f"var_{sfx}", tag="var", bufs=1)
            nc.scalar.square(var, mu)
            nc.vector.scalar_tensor_tensor(out=var, in0=ps_sq, scalar=1.0 / H,
                                           in1=var, op0=ALU.mult,
                                           op1=ALU.subtract)
            nc.scalar.activation(out=var, in_=var, func=AF.Ln, bias=eps_ln)
            nc.scalar.activation(out=var, in_=var, func=AF.Exp, scale=-0.5)

            lnt = []
            for hc in range(HC):
                nc.vector.tensor_sub(a1[hc], a1[hc], mu)
                nc.vector.tensor_mul(a1[hc], a1[hc], var)
                lt = apool.tile([128, BS], BF, name=f"lnt_{sfx}_{hc}", tag="lnt",
                                bufs=4)
                nc.scalar.activation(out=lt, in_=a1[hc], func=AF.Relu,
                                     bias=lnb[:, hc:hc + 1],
                                     scale=lng[:, hc:hc + 1])
                lnt.append(lt)

            p = []
            for vc in range(VC):
                ps = mm_ps([128, BS], f"p_{sfx}_{vc}")
                for k in range(4):
                    nc.tensor.matmul(ps, w2[k][:, vc * 128:(vc + 1) * 128], lnt[k],
                                     start=(k == 0), stop=(k == 3))
                pt = apool.tile([128, BS], BF, name=f"p_{sfx}_{vc}", tag="p", bufs=2)
                nc.scalar.activation(out=pt, in_=ps, func=AF.Tanh,
                                     bias=b2c[:, vc:vc + 1])
                p.append(pt)

            # ---------------- mem row norms (chain-independent) ----------------
            sqn = []
            for bt in range(NBT):
                n2 = apool.tile([128, N], FP, name=f"n2_{sfx}_{bt}", tag="n2",
                                bufs=4)
                for g in range(NGRP):
                    prod = ppool.tile([128, NGS, M], BF, name=f"prodn_{sfx}",
                                      tag="prod", bufs=3)
                    seg = mem[bt][:, g * NGS:(g + 1) * NGS, :]
                    nc.scalar.square(prod, seg)
                    tree_m(n2[:, g * NGS:(g + 1) * NGS], prod)
                nc.scalar.activation(out=n2, in_=n2, func=AF.Ln)
                nc.scalar.activation(out=n2, in_=n2, func=AF.Exp, scale=0.5)
                sqn.append(n2)

            if stage < 2:
                for vc in range(VC):
                    nc.sync.dma_start(out=outT[t, vc * 128:(vc + 1) * 128, :],
                                      in_=p[vc])
                continue

            # ---------------- LSTM (chain starts: needs rT_prev) ----------------
            h = []
            for hc in range(HC):
                gates = []
                for gi in range(4):
                    oc = gi * 4 + hc
                    osl = slice(oc * 128, (oc + 1) * 128)
                    ps = mm_ps([128, BS], f"z_{sfx}_{oc}")
                    nc.tensor.matmul(ps, wih[0][:, osl], p[0], start=True,
                                     stop=False)
                    nc.tensor.matmul(ps, wih[1][:, osl], p[1], start=False,
                                     stop=False)
                    for k in range(4):
                        nc.tensor.matmul(ps, whh[k][:, osl], h0[k], start=False,
                                         stop=False)
                    nc.tensor.matmul(ps, wih[2][:, osl], rT_prev, start=False,
                                     stop=True)
                    gs = apool.tile([128, BS], FP, name=f"g_{sfx}_{oc}", tag="gt",
                                    bufs=4)
                    nc.scalar.activation(out=gs, in_=ps,
                                         func=(AF.Tanh if gi == 2 else AF.Sigmoid),
                                         bias=bzc[:, oc:oc + 1])
                    gates.append(gs)
                gi_, gf_, gg_, go_ = gates
                t2 = apool.tile([128, BS], FP, name=f"ct2_{sfx}_{hc}", tag="ct",
                                bufs=2)
                nc.vector.tensor_mul(t2, gi_, gg_)
                nc.vector.tensor_mul(gf_, gf_, c0[hc])      # gf_ = f*c0
                nc.vector.tensor_add(t2, t2, gf_)           # t2 = c
                nc.scalar.activation(out=t2, in_=t2, func=AF.Tanh)
                ht = apool.tile([128, BS], BF, name=f"h_{sfx}_{hc}", tag="h", bufs=4)
                nc.vector.tensor_mul(ht, go_, t2)
                h.append(ht)

            if stage < 3:
                for k in range(4):
                    nc.sync.dma_start(out=outT[t, k * 128:(k + 1) * 128, :],
                                      in_=h[k])
                continue

            # ---------------- read head ----------------
            ps_or = mm_ps([M + 6, BS], f"or_{sfx}", tag="or", bufs=2)
            for k in range(4):
                nc.tensor.matmul(ps_or, wr_[k], h[k], start=(k == 0), stop=(k == 3))
            ktan = apool.tile([M, BS], FP, name=f"ktan_{sfx}", tag="ktan", bufs=2)
            nc.scalar.activation(out=ktan, in_=ps_or[:M, :], func=AF.Tanh,
                                 bias=brc[:M, :])
            kh6 = apool.tile([6, BS], FP, name=f"kh6_{sfx}", tag="kh6", bufs=2)
            nc.vector.tensor_scalar(out=kh6, in0=ps_or[M:M + 6, :],
                                    scalar1=brc[M:M + 6, :], scalar2=None,
                                    op0=ALU.add)

            if stage < 41:
                nc.sync.dma_start(out=outT[t, 0:M, :], in_=ktan)
                nc.sync.dma_start(out=outT[t, M:M + 6, :], in_=kh6)
                continue

            rT_next = spool.tile([M, BS], BF, name=f"rT_{sfx}", tag="rT", bufs=2)

            for bt in range(NBT):
                bsl = slice(bt * 128, (bt + 1) * 128)
                kT = apool.tile([128, M], BF, name=f"kT_{sfx}_{bt}", tag="kT",
                                bufs=2)
                transpose_to(kT, ktan[:, bsl], f"k_{sfx}_{bt}")
                khT = apool.tile([128, 6], FP, name=f"khT_{sfx}_{bt}", tag="khT",
                                 bufs=2)
                transpose_to(khT, kh6[:, bsl], f"kh_{sfx}_{bt}")

                def sc(nm):
                    return apool.tile([128, 1], FP, name=f"{nm}_{sfx}_{bt}",
                                      tag="sc1", bufs=16)

                def softplus(dst, src):  # ln(1 + exp(x)); head outputs are small
                    nc.scalar.activation(out=dst, in_=src, func=AF.Exp)
                    nc.vector.tensor_scalar(out=dst, in0=dst, scalar1=1.0,
                                            scalar2=None, op0=ALU.add)
                    nc.scalar.activation(out=dst, in_=dst, func=AF.Ln)

                beta = sc("beta")
                softplus(beta, khT[:, 0:1])
                gint = sc("gint")
                nc.scalar.activation(out=gint, in_=khT[:, 1:2], func=AF.Tanh,
                                     scale=0.5)
                nc.vector.tensor_scalar(out=gint, in0=gint, scalar1=0.5,
                                        scalar2=0.5, op0=ALU.mult, op1=ALU.add)
                if stage < 42:
                    nc.sync.dma_start(
                        out=outT[t, bt * 128:(bt + 1) * 128, 0:1], in_=beta)
                    continue
                smx = sc("smx")
                nc.vector.tensor_reduce(out=smx, in_=khT[:, 2:5], axis=AX.X,
                                        op=ALU.max, negate=True)
                s3 = apool.tile([128, 3], FP, name=f"s3_{sfx}_{bt}", tag="s3",
                                bufs=2)
                nc.scalar.activation(out=s3, in_=khT[:, 2:5], func=AF.Exp,
                                     bias=smx)
                ssum = sc("ssum")
                nc.vector.reduce_sum(out=ssum, in_=s3, axis=AX.X)
                nc.vector.reciprocal(out=ssum, in_=ssum)
                nc.vector.tensor_scalar(out=s3, in0=s3, scalar1=ssum,
                                        scalar2=None, op0=ALU.mult)
                gam = sc("gam")
                softplus(gam, khT[:, 5:6])
                nc.vector.tensor_scalar(out=gam, in0=gam, scalar1=1.0,
                                        scalar2=None, op0=ALU.add)
                if stage < 43:
                    nc.sync.dma_start(
                        out=outT[t, bt * 128:(bt + 1) * 128, 0:3], in_=s3)
                    continue
                kn2 = sc("kn2")
                ksq = apool.tile([128, M], FP, name=f"ksq_{sfx}_{bt}", tag="ksq",
                                 bufs=2)
                nc.vector.tensor_mul(ksq, kT, kT)
                nc.vector.reduce_sum(out=kn2, in_=ksq, axis=AX.X)
                nc.scalar.activation(out=kn2, in_=kn2, func=AF.Ln)
                nc.scalar.activation(out=kn2, in_=kn2, func=AF.Exp, scale=0.5)
                if stage < 44:
                    nc.sync.dma_start(
                        out=outT[t, bt * 128:(bt + 1) * 128, 0:1], in_=kn2)
                    continue

                # cosine similarity numerator, then full addressing
                cn = apool.tile([128, N], FP, name=f"cn_{sfx}_{bt}", tag="cn",
                                bufs=2)
                for g in range(NGRP):
                    prod = ppool.tile([128, NGS, M], BF, name=f"prodc_{sfx}",
                                      tag="prod", bufs=3)
                    nc.vector.tensor_mul(prod,
                                         mem[bt][:, g * NGS:(g + 1) * NGS, :],
                                         _bcast_mid(kT, NGS))
                    tree_m(cn[:, g * NGS:(g + 1) * NGS], prod)
                den = apool.tile([128, N], FP, name=f"den_{sfx}_{bt}", tag="den",
                                 bufs=2)
                nc.vector.tensor_scalar(out=den, in0=sqn[bt], scalar1=kn2,
                                        scalar2=EPS, op0=ALU.mult, op1=ALU.add)
                nc.vector.reciprocal(out=den, in_=den)
                nc.vector.tensor_mul(cn, cn, den)
                if stage < 45:
                    nc.sync.dma_start(
                        out=outT[t, bt * 128:(bt + 1) * 128, 0:N], in_=cn)
                    continue
                # wc = softmax(beta * cos)
                nc.vector.tensor_scalar(out=cn, in0=cn, scalar1=beta,
                                        scalar2=None, op0=ALU.mult)
                mx = sc("mx")
                nc.vector.tensor_reduce(out=mx, in_=cn, axis=AX.X, op=ALU.max,
                                        negate=True)
                nc.scalar.activation(out=cn, in_=cn, func=AF.Exp, bias=mx)
                esum = sc("esum")
                nc.vector.reduce_sum(out=esum, in_=cn, axis=AX.X)
                nc.vector.reciprocal(out=esum, in_=esum)
                nc.vector.tensor_scalar(out=cn, in0=cn, scalar1=esum,
                                        scalar2=None, op0=ALU.mult)
                # wg = g*(wc - wprev) + wprev
                nc.vector.tensor_sub(cn, cn, w0[bt])
                nc.vector.tensor_scalar(out=cn, in0=cn, scalar1=gint,
                                        scalar2=None, op0=ALU.mult)
                nc.vector.tensor_add(cn, cn, w0[bt])
                if stage < 46:
                    nc.sync.dma_start(
                        out=outT[t, bt * 128:(bt + 1) * 128, 0:N], in_=cn)
                    continue
                # ws = s0*roll(wg,+1) + s1*wg + s2*roll(wg,-1)
                wmid = apool.tile([128, N], FP, name=f"wmid_{sfx}_{bt}",
                                  tag="wmid", bufs=2)
                nc.vector.tensor_scalar(out=wmid, in0=cn, scalar1=s3[:, 1:2],
                                        scalar2=None, op0=ALU.mult)
                ws = apool.tile([128, N], FP, name=f"ws_{sfx}_{bt}", tag="ws",
                                bufs=2)
                nc.vector.scalar_tensor_tensor(out=ws[:, 1:N], in0=cn[:, 0:N - 1],
                                               scalar=s3[:, 0:1],
                                               in1=wmid[:, 1:N],
                                               op0=ALU.mult, op1=ALU.add)
                nc.vector.scalar_tensor_tensor(out=ws[:, 0:1], in0=cn[:, N - 1:N],
                                               scalar=s3[:, 0:1],
                                               in1=wmid[:, 0:1],
                                               op0=ALU.mult, op1=ALU.add)
                nc.vector.scalar_tensor_tensor(out=wmid[:, 0:N - 1],
                                               in0=cn[:, 1:N],
                                               scalar=s3[:, 2:3],
                                               in1=ws[:, 0:N - 1],
                                               op0=ALU.mult, op1=ALU.add)
                nc.vector.scalar_tensor_tensor(out=wmid[:, N - 1:N],
                                               in0=cn[:, 0:1],
                                               scalar=s3[:, 2:3],
                                               in1=ws[:, N - 1:N],
                                               op0=ALU.mult, op1=ALU.add)
                if stage < 47:
                    nc.sync.dma_start(
                        out=outT[t, bt * 128:(bt + 1) * 128, 0:N], in_=wmid)
                    continue
                # sharpen: w = ws**gamma / (sum + eps)
                nc.scalar.activation(out=wmid, in_=wmid, func=AF.Ln)
                nc.vector.tensor_scalar(out=wmid, in0=wmid, scalar1=gam,
                                        scalar2=None, op0=ALU.mult)
                nc.scalar.activation(out=wmid, in_=wmid, func=AF.Exp)
                wsum = sc("wsum")
                nc.vector.reduce_sum(out=wsum, in_=wmid, axis=AX.X)
                nc.vector.tensor_scalar(out=wsum, in0=wsum, scalar1=EPS,
                                        scalar2=None, op0=ALU.add)
                nc.vector.reciprocal(out=wsum, in_=wsum)
                nc.vector.tensor_scalar(out=wmid, in0=wmid, scalar1=wsum,
                                        scalar2=None, op0=ALU.mult)
                wrb = apool.tile([128, N], BF, name=f"wrb_{sfx}_{bt}", tag="wrb",
                                 bufs=2)
                nc.scalar.copy(out=wrb, in_=wmid)

                if stage < 50:
                    nc.sync.dma_start(
                        out=outT[t, bt * 128:(bt + 1) * 128, 0:N], in_=wmid)
                    continue

                # r = sum_n w[b,n] * mem[b,n,:]
                rp = apool.tile([128, NGRP, M], FP, name=f"rp_{sfx}_{bt}",
                                tag="rp", bufs=1)
                for g in range(NGRP):
                    prod = ppool.tile([128, NGS, M], BF, name=f"prodr_{sfx}",
                                      tag="prod", bufs=3)
                    wseg = wrb[:, g * NGS:(g + 1) * NGS]
                    nc.vector.tensor_mul(prod,
                                         mem[bt][:, g * NGS:(g + 1) * NGS, :],
                                         _bcast_inner(wseg, M))
                    tree_n(rp[:, g:g + 1, :], prod)
                st = 1
                while st < NGRP:
                    for g0 in range(0, NGRP, 2 * st):
                        nc.vector.tensor_add(rp[:, g0, :], rp[:, g0, :],
                                             rp[:, g0 + st, :])
                    st *= 2
                transpose_to(rT_next[:, bsl], rp[:, 0, :], f"r_{sfx}_{bt}")

            if stage < 41:
                continue
            if stage < 99:
                if stage >= 50:
                    nc.sync.dma_start(out=outT[t, 0:M, :], in_=rT_next)
                rT_prev = rT_next if stage >= 50 else rT_prev
                continue

            # ---------------- output projection ----------------
            for ec in range(EC):
                esl = slice(ec * 128, (ec + 1) * 128)
                ps = mm_ps([128, BS], f"o_{sfx}_{ec}")
                for k in range(4):
                    nc.tensor.matmul(ps, wo[k][:, esl], h[k], start=(k == 0),
                                     stop=False)
                nc.tensor.matmul(ps, wo[4][:, esl], rT_next, start=False,
                                 stop=True)
                os_ = apool.tile([128, BS], FP, name=f"os_{sfx}_{ec}", tag="os",
                                 bufs=2)
                nc.scalar.activation(out=os_, in_=ps, func=AF.Tanh, scale=0.5,
                                     bias=boc[:, ec:ec + 1])
                nc.vector.tensor_scalar(out=os_, in0=os_, scalar1=0.5,
                                        scalar2=0.5, op0=ALU.mult, op1=ALU.add)
                nc.sync.dma_start(out=outT[t, esl, :], in_=os_)

            rT_prev = rT_next

    nc.compile()
    return nc


_CACHE = {}
LAST = {}


def _get_nc():
    if "nc" not in _CACHE:
        _CACHE["nc"] = build_nc()
    return _CACHE["nc"]


def host_prep(inputs, W1, b1, lng, lnb, W2, b2, Wih, Whh, bih, bhh,
              Wr, br, Ww, bw, Wo, bo, mem0, read0, wr0, ww0, h0, c0):
    f32 = np.float32
    inputs, W1, W2, Wih, Whh, Wr, Wo = [np.asarray(a, f32) for a in
                                        (inputs, W1, W2, Wih, Whh, Wr, Wo)]

    def percol(v, cols):   # [T, 128*cols] -> [T, 128, cols] column-major chunks
        return np.ascontiguousarray(
            np.asarray(v, f32).reshape(T, cols, 128).transpose(0, 2, 1))

    bf = ml_dtypes.bfloat16
    xT_full = np.ascontiguousarray(inputs.transpose(0, 2, 1))      # [T, E, B]
    w1t = np.ascontiguousarray(W1.transpose(0, 2, 1))              # [T, E, H]
    w2t = np.ascontiguousarray(W2.transpose(0, 2, 1)).astype(bf)   # [T, H, V]
    wiht = np.ascontiguousarray(Wih.transpose(0, 2, 1)).astype(bf)
    whht = np.ascontiguousarray(Whh.transpose(0, 2, 1)).astype(bf)
    wrt = np.ascontiguousarray(Wr.transpose(0, 2, 1)).astype(bf)   # [T, H, 70]
    wot = np.ascontiguousarray(Wo.transpose(0, 2, 1)).astype(bf)   # [T, 576, E]
    h0t_full = np.asarray(h0, f32).transpose(0, 2, 1).astype(bf)
    c0t_full = np.ascontiguousarray(np.asarray(c0, f32).transpose(0, 2, 1))
    r0t_full = np.asarray(read0, f32)[T - 1].T.astype(bf)          # [M, B]
    wr0_full = np.asarray(wr0, f32)
    mem0_full = np.asarray(mem0).astype(ml_dtypes.bfloat16)
    bz = np.asarray(bih, f32) + np.asarray(bhh, f32)

    common = dict(
        w1t=w1t, w2t=w2t, wiht=wiht, whht=whht, wrt=wrt, wot=wot,
        b1c=percol(b1, HC), lngc=percol(lng, HC), lnbc=percol(lnb, HC),
        b2c=percol(b2, VC), bzc=percol(bz, ZC), bzch=percol(0.5 * bz, ZC),
        brc=np.ascontiguousarray(np.asarray(br, f32).reshape(T, M + 6, 1)),
        boc=percol(bo, EC),
    )
    in_maps = []
    for ci in range(NCORES):
        bsl = slice(ci * BS, (ci + 1) * BS)
        in_maps.append(dict(
            common,
            xT=np.ascontiguousarray(xT_full[:, :, bsl]),
            h0t=np.ascontiguousarray(h0t_full[:, :, bsl]),
            c0t=np.ascontiguousarray(c0t_full[:, :, bsl]),
            r0t=np.ascontiguousarray(r0t_full[:, bsl]),
            wr0=np.ascontiguousarray(wr0_full[:, bsl, :]),
            mem0=np.ascontiguousarray(mem0_full[:, bsl]),
        ))

    return in_maps


def kernel(**inputs):
    in_maps = host_prep(**inputs)
    nc = _get_nc()
    import os
    trace = os.environ.get("BASS_TRACE", "") not in ("", "0")
    res = run_bass_kernel_spmd(nc, in_maps, list(range(NCORES)), trace=trace)
    LAST["exec_time_ns"] = res.exec_time_ns
    LAST["results"] = res
    out = np.concatenate(
        [np.transpose(r["outT"], (0, 2, 1)) for r in res.results], axis=1)
    return np.ascontiguousarray(out.astype(np.float32))


# revision 25
# speedup vs baseline: 1.1556x; 1.0039x over previous
"""Trainium2 Bass kernel for nn_CM_NTM_29566554866014 (scatter_memory).

Sharding: pure batch data-parallelism across 8 NeuronCores (B=2048 -> 256/core).
Small parameters replicated. The cross-NTM loop (T=4) is sequential but
batch-local, so each core runs all 4 steps on its batch shard independently.
No collectives.

Key structural facts used (verified against the reference math):
  * The write head (Ww/bw/ww0) and the memory erase/add update are dead code:
    `mem` is reassigned to `mem0[i+1]` each iteration and outputs depend only
    on h and r. They are therefore not computed.
  * Only read0[T-1] is consumed.
  * Per-step state (mem0/h0/c0/wr0) are fresh inputs each step; the only
    sequential dependency across steps is the read vector r.

Layouts:
  * Matmul stack is feature-major ([feat, batch] with feat on partitions) so
    contractions run on the tensor engine with host-pre-transposed weights.
  * NTM addressing is batch-major ([batch, N] / [batch, N, M]) so softmax /
    shift / sharpen are free-dim ops. mem0 is uploaded bf16 (SBUF fit + DVE),
    products accumulate to fp32.
"""

import numpy as np
import ml_dtypes
from contextlib import ExitStack

import concourse.bass as bass
import concourse.tile as tile
from concourse import bacc
from concourse import mybir
from concourse.bass_utils import run_bass_kernel_spmd
from concourse.masks import make_identity

AF = mybir.ActivationFunctionType
ALU = mybir.AluOpType
AX = mybir.AxisListType
FP = mybir.dt.float32
BF = mybir.dt.bfloat16

T, E, V, H, N, M, B = 4, 512, 256, 512, 128, 64, 2048
NCORES = 8
BS = B // NCORES      # 256 batch rows per core
NBT = BS // 128       # 2 batch tiles
HC = H // 128         # 4
EC = E // 128         # 4
VC = V // 128         # 2
ZC = (4 * H) // 128   # 16
NGRP = 2              # n-groups for mem scratch
NGS = N // NGRP       # 16
EPS = 1e-16


def _bcast_inner(ap, count):
    """View `ap` ([P, F]) as [P, F, count] with a stride-0 innermost dim."""
    return bass.AP(tensor=ap.tensor, offset=ap.offset,
                   ap=[*ap.ap, [0, count]])


def _bcast_mid(ap, count):
    """View `ap` ([P, F]) as [P, count, F] with a stride-0 middle dim."""
    return bass.AP(tensor=ap.tensor, offset=ap.offset,
                   ap=[ap.ap[0], [0, count], ap.ap[1]])


def _swap_free(ap):
    """Swap the two free dims of a 3-dim AP ([P, A, B] -> [P, B, A])."""
    return bass.AP(tensor=ap.tensor, offset=ap.offset,
                   ap=[ap.ap[0], ap.ap[2], ap.ap[1]])


def build_nc(stage=None):
    import os
    if stage is None:
        stage = int(os.environ.get("NTM_STAGE", "99"))
    nc = bacc.Bacc()
    d = {}

    def din(name, shape, dt=FP):
        d[name] = nc.dram_tensor(name, list(shape), dt, kind="ExternalInput")

    din("xT",   (T, E, BS))
    din("w1t",  (T, E, H))
    din("w2t",  (T, H, V), BF)
    din("wiht", (T, V + M, 4 * H), BF)
    din("whht", (T, H, 4 * H), BF)
    din("wrt",  (T, H, M + 6), BF)
    din("wot",  (T, H + M, E), BF)
    din("h0t",  (T, H, BS), BF)
    din("c0t",  (T, H, BS))
    din("r0t",  (M, BS), BF)
    din("wr0",  (T, BS, N))
    din("mem0", (T, BS, N, M), BF)
    din("b1c",  (T, 128, HC))
    din("lngc", (T, 128, HC))
    din("lnbc", (T, 128, HC))
    din("b2c",  (T, 128, VC))
    din("bzc",  (T, 128, ZC))
    din("bzch", (T, 128, ZC))
    din("brc",  (T, M + 6, 1))
    din("boc",  (T, 128, EC))
    outT = nc.dram_tensor("outT", [T, E, BS], FP, kind="ExternalOutput")

    with tile.TileContext(nc) as tc, ExitStack() as ctx:
        singles = ctx.enter_context(tc.tile_pool(name="singles", bufs=1))
        wpool = ctx.enter_context(tc.tile_pool(name="wpool", bufs=1))
        spool = ctx.enter_context(tc.tile_pool(name="spool", bufs=1))
        apool = ctx.enter_context(tc.tile_pool(name="apool", bufs=1))
        mpool = ctx.enter_context(tc.tile_pool(name="mpool", bufs=1))
        ppool = ctx.enter_context(tc.tile_pool(name="ppool", bufs=1))
        pmm = ctx.enter_context(tc.tile_pool(name="pmm", bufs=1, space="PSUM"))

        ones_t = singles.tile([128, 128], FP, name="ones_t")
        nc.vector.memset(ones_t, 1.0)
        ident = singles.tile([128, 128], FP, name="ident")
        make_identity(nc, ident)
        eps_ln = singles.tile([128, 1], FP, name="eps_ln")
        nc.vector.memset(eps_ln, 1e-5)

        def mm_ps(shape, name, tag="mm", bufs=4):
            return pmm.tile(shape, FP, name=name, tag=tag, bufs=bufs)

        def transpose_to(dst_ap, src_ap, name):
            """PE-transpose src ([p, f], f<=128) into SBUF dst ([f, p])."""
            p, f = src_ap.shape
            ps = mm_ps([f, p], f"tp_{name}", tag="tp", bufs=2)
            nc.tensor.transpose(ps, src_ap, ident[:p, :p])
            nc.scalar.copy(out=dst_ap, in_=ps)

        def tree_m(dst2d, prod, eng=None, tag="trm"):
            """Sum prod [128, G, M(=64)] over innermost m into dst2d [128, G]
            fp32 via pairwise bf16 adds (DVE 2x mode)."""
            eng = eng or nc.vector
            G = prod.shape[1]
            s1 = ppool.tile([128, G, M // 2], BF, name="trm", tag=tag, bufs=3)
            eng.tensor_add(s1, prod[:, :, 0:M // 2], prod[:, :, M // 2:M])
            w = M // 2
            while w > 2:
                hw = w // 2
                eng.tensor_add(s1[:, :, 0:hw], s1[:, :, 0:hw],
                               s1[:, :, hw:w])
                w = hw
            dst3 = bass.AP(tensor=dst2d.tensor, offset=dst2d.offset,
                           ap=[*dst2d.ap, [1, 1]])
            eng.tensor_add(dst3, s1[:, :, 0:1], s1[:, :, 1:2])

        def tree_n(dst3d, prod):
            """Sum prod [128, G(=64), M] over axis 1 into dst3d [128, 1, M]
            fp32 via pairwise bf16 adds on contiguous halves."""
            G = prod.shape[1]
            s1 = ppool.tile([128, G // 2, M], BF, name="trn", tag="trn", bufs=3)
            nc.vector.tensor_add(s1, prod[:, 0:G // 2, :], prod[:, G // 2:G, :])
            w = G // 2
            while w > 2:
                hw = w // 2
                nc.vector.tensor_add(s1[:, 0:hw, :], s1[:, 0:hw, :],
                                     s1[:, hw:w, :])
                w = hw
            nc.vector.tensor_add(dst3d, s1[:, 0:1, :], s1[:, 1:2, :])

        rT_prev = None
        for t in range(T):
            sfx = f"t{t}"
            # ---------------- loads ----------------
            w1 = [wpool.tile([128, H], FP, name=f"w1_{sfx}_{k}", tag="w1",
                             bufs=4) for k in range(4)]
            for k in range(4):
                nc.sync.dma_start(out=w1[k], in_=d["w1t"][t, k * 128:(k + 1) * 128, :])
            w2 = [wpool.tile([128, V], BF, name=f"w2_{sfx}_{k}", tag="w2",
                             bufs=4) for k in range(4)]
            for k in range(4):
                nc.sync.dma_start(out=w2[k], in_=d["w2t"][t, k * 128:(k + 1) * 128, :])
            wih = []
            for k, ksz in enumerate((128, 128, 64)):
                wt = wpool.tile([ksz, 4 * H], BF, name=f"wih_{sfx}_{k}", tag="wih",
                                bufs=3)
                nc.sync.dma_start(out=wt, in_=d["wiht"][t, k * 128:k * 128 + ksz, :])
                wih.append(wt)
            whh = [wpool.tile([128, 4 * H], BF, name=f"whh_{sfx}_{k}", tag="whh",
                              bufs=4) for k in range(4)]
            for k in range(4):
                nc.sync.dma_start(out=whh[k], in_=d["whht"][t, k * 128:(k + 1) * 128, :])
            wr_ = [wpool.tile([128, M + 6], BF, name=f"wr_{sfx}_{k}", tag="wr",
                              bufs=4) for k in range(4)]
            for k in range(4):
                nc.sync.dma_start(out=wr_[k], in_=d["wrt"][t, k * 128:(k + 1) * 128, :])
            wo = []
            for k, ksz in enumerate((128, 128, 128, 128, 64)):
                wt = wpool.tile([ksz, E], BF, name=f"wo_{sfx}_{k}", tag="wo", bufs=5)
                nc.sync.dma_start(out=wt, in_=d["wot"][t, k * 128:k * 128 + ksz, :])
                wo.append(wt)

            xT = [spool.tile([128, BS], FP, name=f"xT_{sfx}_{k}", tag="xT",
                             bufs=4) for k in range(4)]
            h0 = [spool.tile([128, BS], BF, name=f"h0_{sfx}_{k}", tag="h0",
                             bufs=4) for k in range(4)]
            c0 = [spool.tile([128, BS], FP, name=f"c0_{sfx}_{k}", tag="c0",
                             bufs=4) for k in range(4)]
            for k in range(4):
                nc.sync.dma_start(out=xT[k], in_=d["xT"][t, k * 128:(k + 1) * 128, :])
                nc.sync.dma_start(out=h0[k], in_=d["h0t"][t, k * 128:(k + 1) * 128, :])
                nc.sync.dma_start(out=c0[k], in_=d["c0t"][t, k * 128:(k + 1) * 128, :])

            b1c = spool.tile([128, HC], FP, name=f"b1c_{sfx}", tag="b1c", bufs=2)
            lng = spool.tile([128, HC], FP, name=f"lng_{sfx}", tag="lng", bufs=2)
            lnb = spool.tile([128, HC], FP, name=f"lnb_{sfx}", tag="lnb", bufs=2)
            b2c = spool.tile([128, VC], FP, name=f"b2c_{sfx}", tag="b2c", bufs=2)
            bzc = spool.tile([128, ZC], FP, name=f"bzc_{sfx}", tag="bzc", bufs=2)
            bzch = spool.tile([128, ZC], FP, name=f"bzch_{sfx}", tag="bzch", bufs=2)
            brc = spool.tile([M + 6, 1], FP, name=f"brc_{sfx}", tag="brc", bufs=2)
            boc = spool.tile([128, EC], FP, name=f"boc_{sfx}", tag="boc", bufs=2)
            nc.sync.dma_start(out=b1c, in_=d["b1c"][t])
            nc.sync.dma_start(out=lng, in_=d["lngc"][t])
            nc.sync.dma_start(out=lnb, in_=d["lnbc"][t])
            nc.sync.dma_start(out=b2c, in_=d["b2c"][t])
            nc.sync.dma_start(out=bzc, in_=d["bzc"][t])
            nc.sync.dma_start(out=bzch, in_=d["bzch"][t])
            nc.sync.dma_start(out=brc, in_=d["brc"][t])
            nc.sync.dma_start(out=boc, in_=d["boc"][t])

            mem = []
            w0 = []
            for bt in range(NBT):
                mt = mpool.tile([128, N, M], BF, name=f"mem_{sfx}_{bt}", tag="mem",
                                bufs=3)
                nc.sync.dma_start(out=mt, in_=d["mem0"][t, bt * 128:(bt + 1) * 128])
                mem.append(mt)
                wt = spool.tile([128, N], FP, name=f"w0_{sfx}_{bt}", tag="w0", bufs=4)
                nc.sync.dma_start(out=wt, in_=d["wr0"][t, bt * 128:(bt + 1) * 128, :])
                w0.append(wt)

            if t == 0:
                rT_prev = spool.tile([M, BS], BF, name="r0T", tag="rT", bufs=2)
                nc.sync.dma_start(out=rT_prev, in_=d["r0t"][:, :])

            # ---------------- input projection + LN + p ----------------
            a1 = []
            for hc in range(HC):
                ps = mm_ps([128, BS], f"a1_{sfx}_{hc}")
                for k in range(4):
                    nc.tensor.matmul(ps, w1[k][:, hc * 128:(hc + 1) * 128], xT[k],
                                     start=(k == 0), stop=(k == 3))
                a1s = apool.tile([128, BS], FP, name=f"a1s_{sfx}_{hc}", tag="a1",
                                 bufs=4)
                nc.vector.tensor_scalar(out=a1s, in0=ps,
                                        scalar1=b1c[:, hc:hc + 1], scalar2=None,
                                        op0=ALU.add)
                a1.append(a1s)

            ps_sum = mm_ps([128, BS], f"sums_{sfx}")
            for k in range(4):
                nc.tensor.matmul(ps_sum, ones_t, a1[k], start=(k == 0),
                                 stop=(k == 3))
            ps_sq = mm_ps([128, BS], f"sumsq_{sfx}")
            for k in range(4):
                sq = ppool.tile([128, BS], FP, name=f"sq_{sfx}_{k}", tag="sq",
                                bufs=2)
                nc.scalar.square(sq, a1[k])
                nc.tensor.matmul(ps_sq, ones_t, sq, start=(k == 0), stop=(k == 3))

            mu = apool.tile([128, BS], FP, name=f"mu_{sfx}", tag="mu", bufs=1)
            nc.vector.tensor_scalar(out=mu, in0=ps_sum, scalar1=1.0 / H,
                                    scalar2=None, op0=ALU.mult)
            var = apool.tile([128, BS], FP, name=f"var_{sfx}", tag="var", bufs=1)
            nc.scalar.square(var, mu)
            nc.vector.scalar_tensor_tensor(out=var, in0=ps_sq, scalar=1.0 / H,
                                           in1=var, op0=ALU.mult,
                                           op1=ALU.subtract)
            nc.scalar.activation(out=var, in_=var, func=AF.Ln, bias=eps_ln)
            nc.scalar.activation(out=var, in_=var, func=AF.Exp, scale=-0.5)

            lnt = []
            for hc in range(HC):
                nc.vector.tensor_sub(a1[hc], a1[hc], mu)
                nc.vector.tensor_mul(a1[hc], a1[hc], var)
                lt = apool.tile([128, BS], BF, name=f"lnt_{sfx}_{hc}", tag="lnt",
                                bufs=4)
                nc.scalar.activation(out=lt, in_=a1[hc], func=AF.Relu,
                                     bias=lnb[:, hc:hc + 1],
                                     scale=lng[:, hc:hc + 1])
                lnt.append(lt)

            p = []
            for vc in range(VC):
                ps = mm_ps([128, BS], f"p_{sfx}_{vc}")
                for k in range(4):
                    nc.tensor.matmul(ps, w2[k][:, vc * 128:(vc + 1) * 128], lnt[k],
                                     start=(k == 0), stop=(k == 3))
                pt = apool.tile([128, BS], BF, name=f"p_{sfx}_{vc}", tag="p", bufs=2)
                nc.scalar.activation(out=pt, in_=ps, func=AF.Tanh,
                                     bias=b2c[:, vc:vc + 1])
                p.append(pt)

            # ---------------- mem row norms (chain-independent) ----------------
            sqn = []
            for bt in range(NBT):
                n2 = apool.tile([128, N], FP, name=f"n2_{sfx}_{bt}", tag="n2",
                                bufs=4)
                for g in range(NGRP):
                    prod = ppool.tile([128, NGS, M], BF, name=f"prodn_{sfx}",
                                      tag="prod", bufs=3)
                    seg = mem[bt][:, g * NGS:(g + 1) * NGS, :]
                    nc.scalar.square(prod, seg)
                    tree_m(n2[:, g * NGS:(g + 1) * NGS], prod)
                nc.scalar.activation(out=n2, in_=n2, func=AF.Ln)
                nc.scalar.activation(out=n2, in_=n2, func=AF.Exp, scale=0.5)
                sqn.append(n2)

            if stage < 2:
                for vc in range(VC):
                    nc.sync.dma_start(out=outT[t, vc * 128:(vc + 1) * 128, :],
                                      in_=p[vc])
                continue

            # ---------------- LSTM (chain starts: needs rT_prev) ----------------
            h = []
            for hc in range(HC):
                gates = []
                for gi in range(4):
                    oc = gi * 4 + hc
                    osl = slice(oc * 128, (oc + 1) * 128)
                    ps = mm_ps([128, BS], f"z_{sfx}_{oc}")
                    nc.tensor.matmul(ps, wih[0][:, osl], p[0], start=True,
                                     stop=False)
                    nc.tensor.matmul(ps, wih[1][:, osl], p[1], start=False,
                                     stop=False)
                    for k in range(4):
                        nc.tensor.matmul(ps, whh[k][:, osl], h0[k], start=False,
                                         stop=False)
                    nc.tensor.matmul(ps, wih[2][:, osl], rT_prev, start=False,
                                     stop=True)
                    gs = apool.tile([128, BS], FP, name=f"g_{sfx}_{oc}", tag="gt",
                                    bufs=4)
                    nc.scalar.activation(out=gs, in_=ps,
                                         func=(AF.Tanh if gi == 2 else AF.Sigmoid),
                                         bias=bzc[:, oc:oc + 1])
                    gates.append(gs)
                gi_, gf_, gg_, go_ = gates
                t2 = apool.tile([128, BS], FP, name=f"ct2_{sfx}_{hc}", tag="ct",
                                bufs=2)
                nc.vector.tensor_mul(t2, gi_, gg_)
                nc.vector.tensor_mul(gf_, gf_, c0[hc])      # gf_ = f*c0
                nc.vector.tensor_add(t2, t2, gf_)           # t2 = c
                nc.scalar.activation(out=t2, in_=t2, func=AF.Tanh)
                ht = apool.tile([128, BS], BF, name=f"h_{sfx}_{hc}", tag="h", bufs=4)
                nc.vector.tensor_mul(ht, go_, t2)
                h.append(ht)

            if stage < 3:
                for k in range(4):
                    nc.sync.dma_start(out=outT[t, k * 128:(k + 1) * 128, :],
                                      in_=h[k])
                continue

            # ---------------- read head ----------------
            ps_or = mm_ps([M + 6, BS], f"or_{sfx}", tag="or", bufs=2)
            for k in range(4):
                nc.tensor.matmul(ps_or, wr_[k], h[k], start=(k == 0), stop=(k == 3))
            ktan = apool.tile([M, BS], FP, name=f"ktan_{sfx}", tag="ktan", bufs=2)
            nc.scalar.activation(out=ktan, in_=ps_or[:M, :], func=AF.Tanh,
                                 bias=brc[:M, :])
            kh6 = apool.tile([6, BS], FP, name=f"kh6_{sfx}", tag="kh6", bufs=2)
            nc.vector.tensor_scalar(out=kh6, in0=ps_or[M:M + 6, :],
                                    scalar1=brc[M:M + 6, :], scalar2=None,
                                    op0=ALU.add)

            if stage < 41:
                nc.sync.dma_start(out=outT[t, 0:M, :], in_=ktan)
                nc.sync.dma_start(out=outT[t, M:M + 6, :], in_=kh6)
                continue

            rT_next = spool.tile([M, BS], BF, name=f"rT_{sfx}", tag="rT", bufs=2)

            for bt in range(NBT):
                bsl = slice(bt * 128, (bt + 1) * 128)
                kT = apool.tile([128, M], BF, name=f"kT_{sfx}_{bt}", tag="kT",
                                bufs=2)
                transpose_to(kT, ktan[:, bsl], f"k_{sfx}_{bt}")
                khT = apool.tile([128, 6], FP, name=f"khT_{sfx}_{bt}", tag="khT",
                                 bufs=2)
                transpose_to(khT, kh6[:, bsl], f"kh_{sfx}_{bt}")

                def sc(nm):
                    return apool.tile([128, 1], FP, name=f"{nm}_{sfx}_{bt}",
                                      tag="sc1", bufs=16)

                def softplus(dst, src):  # ln(1 + exp(x)); head outputs are small
                    nc.scalar.activation(out=dst, in_=src, func=AF.Exp)
                    nc.vector.tensor_scalar(out=dst, in0=dst, scalar1=1.0,
                                            scalar2=None, op0=ALU.add)
                    nc.scalar.activation(out=dst, in_=dst, func=AF.Ln)

                beta = sc("beta")
                softplus(beta, khT[:, 0:1])
                gint = sc("gint")
                # sigmoid via exp+recip keeps the head in the exp/ln ACT set
                nc.scalar.activation(out=gint, in_=khT[:, 1:2], func=AF.Exp,
                                     scale=-1.0)
                nc.vector.tensor_scalar(out=gint, in0=gint, scalar1=1.0,
                                        scalar2=None, op0=ALU.add)
                nc.vector.reciprocal(out=gint, in_=gint)
                if stage < 42:
                    nc.sync.dma_start(
                        out=outT[t, bt * 128:(bt + 1) * 128, 0:1], in_=beta)
                    continue
                smx = sc("smx")
                nc.vector.tensor_reduce(out=smx, in_=khT[:, 2:5], axis=AX.X,
                                        op=ALU.max, negate=True)
                s3 = apool.tile([128, 3], FP, name=f"s3_{sfx}_{bt}", tag="s3",
                                bufs=2)
                nc.scalar.activation(out=s3, in_=khT[:, 2:5], func=AF.Exp,
                                     bias=smx)
                ssum = sc("ssum")
                nc.vector.reduce_sum(out=ssum, in_=s3, axis=AX.X)
                nc.vector.reciprocal(out=ssum, in_=ssum)
                nc.vector.tensor_scalar(out=s3, in0=s3, scalar1=ssum,
                                        scalar2=None, op0=ALU.mult)
                gam = sc("gam")
                softplus(gam, khT[:, 5:6])
                nc.vector.tensor_scalar(out=gam, in0=gam, scalar1=1.0,
                                        scalar2=None, op0=ALU.add)
                if stage < 43:
                    nc.sync.dma_start(
                        out=outT[t, bt * 128:(bt + 1) * 128, 0:3], in_=s3)
                    continue
                kn2 = sc("kn2")
                ksq = apool.tile([128, M], FP, name=f"ksq_{sfx}_{bt}", tag="ksq",
                                 bufs=2)
                nc.vector.tensor_mul(ksq, kT, kT)
                nc.vector.reduce_sum(out=kn2, in_=ksq, axis=AX.X)
                nc.scalar.activation(out=kn2, in_=kn2, func=AF.Ln)
                nc.scalar.activation(out=kn2, in_=kn2, func=AF.Exp, scale=0.5)
                if stage < 44:
                    nc.sync.dma_start(
                        out=outT[t, bt * 128:(bt + 1) * 128, 0:1], in_=kn2)
                    continue

                # cosine similarity numerator, then full addressing
                cn = apool.tile([128, N], FP, name=f"cn_{sfx}_{bt}", tag="cn",
                                bufs=2)
                for g in range(NGRP):
                    prod = ppool.tile([128, NGS, M], BF, name=f"prodc_{sfx}",
                                      tag="prod", bufs=3)
                    nc.vector.tensor_mul(prod,
                                         mem[bt][:, g * NGS:(g + 1) * NGS, :],
                                         _bcast_mid(kT, NGS))
                    tree_m(cn[:, g * NGS:(g + 1) * NGS], prod)
                den = apool.tile([128, N], FP, name=f"den_{sfx}_{bt}", tag="den",
                                 bufs=2)
                nc.vector.tensor_scalar(out=den, in0=sqn[bt], scalar1=kn2,
                                        scalar2=EPS, op0=ALU.mult, op1=ALU.add)
                nc.vector.reciprocal(out=den, in_=den)
                nc.vector.tensor_mul(cn, cn, den)
                if stage < 45:
                    nc.sync.dma_start(
                        out=outT[t, bt * 128:(bt + 1) * 128, 0:N], in_=cn)
                    continue
                # wc = softmax(beta * cos)
                nc.vector.tensor_scalar(out=cn, in0=cn, scalar1=beta,
                                        scalar2=None, op0=ALU.mult)
                mx = sc("mx")
                nc.vector.tensor_reduce(out=mx, in_=cn, axis=AX.X, op=ALU.max,
                                        negate=True)
                nc.scalar.activation(out=cn, in_=cn, func=AF.Exp, bias=mx)
                esum = sc("esum")
                nc.vector.reduce_sum(out=esum, in_=cn, axis=AX.X)
                nc.vector.reciprocal(out=esum, in_=esum)
                nc.vector.tensor_scalar(out=cn, in0=cn, scalar1=esum,
                                        scalar2=None, op0=ALU.mult)
                # wg = g*(wc - wprev) + wprev
                nc.vector.tensor_sub(cn, cn, w0[bt])
                nc.vector.tensor_scalar(out=cn, in0=cn, scalar1=gint,
                                        scalar2=None, op0=ALU.mult)
                nc.vector.tensor_add(cn, cn, w0[bt])
                if stage < 46:
                    nc.sync.dma_start(
                        out=outT[t, bt * 128:(bt + 1) * 128, 0:N], in_=cn)
                    continue
                # ws = s0*roll(wg,+1) + s1*wg + s2*roll(wg,-1)
                wmid = apool.tile([128, N], FP, name=f"wmid_{sfx}_{bt}",
                                  tag="wmid", bufs=2)
                nc.vector.tensor_scalar(out=wmid, in0=cn, scalar1=s3[:, 1:2],
                                        scalar2=None, op0=ALU.mult)
                ws = apool.tile([128, N], FP, name=f"ws_{sfx}_{bt}", tag="ws",
                                bufs=2)
                nc.vector.scalar_tensor_tensor(out=ws[:, 1:N], in0=cn[:, 0:N - 1],
                                               scalar=s3[:, 0:1],
                                               in1=wmid[:, 1:N],
                                               op0=ALU.mult, op1=ALU.add)
                nc.vector.scalar_tensor_tensor(out=ws[:, 0:1], in0=cn[:, N - 1:N],
                                               scalar=s3[:, 0:1],
                                               in1=wmid[:, 0:1],
                                               op0=ALU.mult, op1=ALU.add)
                nc.vector.scalar_tensor_tensor(out=wmid[:, 0:N - 1],
                                               in0=cn[:, 1:N],
                                               scalar=s3[:, 2:3],
                                               in1=ws[:, 0:N - 1],
                                               op0=ALU.mult, op1=ALU.add)
                nc.vector.scalar_tensor_tensor(out=wmid[:, N - 1:N],
                                               in0=cn[:, 0:1],
                                               scalar=s3[:, 2:3],
                                               in1=ws[:, N - 1:N],
                                               op0=ALU.mult, op1=ALU.add)
                if stage < 47:
                    nc.sync.dma_start(
                        out=outT[t, bt * 128:(bt + 1) * 128, 0:N], in_=wmid)
                    continue
                # sharpen: w = ws**gamma / (sum + eps)
                nc.scalar.activation(out=wmid, in_=wmid, func=AF.Ln)
                nc.vector.tensor_scalar(out=wmid, in0=wmid, scalar1=gam,
                                        scalar2=None, op0=ALU.mult)
                nc.scalar.activation(out=wmid, in_=wmid, func=AF.Exp)
                wsum = sc("wsum")
                nc.vector.reduce_sum(out=wsum, in_=wmid, axis=AX.X)
                nc.vector.tensor_scalar(out=wsum, in0=wsum, scalar1=EPS,
                                        scalar2=None, op0=ALU.add)
                nc.vector.reciprocal(out=wsum, in_=wsum)
                nc.vector.tensor_scalar(out=wmid, in0=wmid, scalar1=wsum,
                                        scalar2=None, op0=ALU.mult)
                wrb = apool.tile([128, N], BF, name=f"wrb_{sfx}_{bt}", tag="wrb",
                                 bufs=2)
                nc.scalar.copy(out=wrb, in_=wmid)

                if stage < 50:
                    nc.sync.dma_start(
                        out=outT[t, bt * 128:(bt + 1) * 128, 0:N], in_=wmid)
                    continue

                # r = sum_n w[b,n] * mem[b,n,:]
                rp = apool.tile([128, NGRP, M], FP, name=f"rp_{sfx}_{bt}",
                                tag="rp", bufs=1)
                for g in range(NGRP):
                    prod = ppool.tile([128, NGS, M], BF, name=f"prodr_{sfx}",
                                      tag="prod", bufs=3)
                    wseg = wrb[:, g * NGS:(g + 1) * NGS]
                    nc.vector.tensor_mul(prod,
                                         mem[bt][:, g * NGS:(g + 1) * NGS, :],
                                         _bcast_inner(wseg, M))
                    tree_n(rp[:, g:g + 1, :], prod)
                st = 1
                while st < NGRP:
                    for g0 in range(0, NGRP, 2 * st):
                        nc.vector.tensor_add(rp[:, g0, :], rp[:, g0, :],
                                             rp[:, g0 + st, :])
                    st *= 2
                transpose_to(rT_next[:, bsl], rp[:, 0, :], f"r_{sfx}_{bt}")

            if stage < 41:
                continue
            if stage < 99:
                if stage >= 50:
                    nc.sync.dma_start(out=outT[t, 0:M, :], in_=rT_next)
                rT_prev = rT_next if stage >= 50 else rT_prev
                continue

            # ---------------- output projection ----------------
            for ec in range(EC):
                esl = slice(ec * 128, (ec + 1) * 128)
                ps = mm_ps([128, BS], f"o_{sfx}_{ec}")
                for k in range(4):
                    nc.tensor.matmul(ps, wo[k][:, esl], h[k], start=(k == 0),
                                     stop=False)
                nc.tensor.matmul(ps, wo[4][:, esl], rT_next, start=False,
                                 stop=True)
                os_ = apool.tile([128, BS], FP, name=f"os_{sfx}_{ec}", tag="os",
                                 bufs=2)
                nc.scalar.activation(out=os_, in_=ps, func=AF.Tanh, scale=0.5,
                                     bias=boc[:, ec:ec + 1])
                nc.vector.tensor_scalar(out=os_, in0=os_, scalar1=0.5,
                                        scalar2=0.5, op0=ALU.mult, op1=ALU.add)
                nc.sync.dma_start(out=outT[t, esl, :], in_=os_)

            rT_prev = rT_next

    nc.compile()
    return nc


_CACHE = {}
LAST = {}


def _get_nc():
    if "nc" not in _CACHE:
        _CACHE["nc"] = build_nc()
    return _CACHE["nc"]


def host_prep(inputs, W1, b1, lng, lnb, W2, b2, Wih, Whh, bih, bhh,
              Wr, br, Ww, bw, Wo, bo, mem0, read0, wr0, ww0, h0, c0):
    f32 = np.float32
    inputs, W1, W2, Wih, Whh, Wr, Wo = [np.asarray(a, f32) for a in
                                        (inputs, W1, W2, Wih, Whh, Wr, Wo)]

    def percol(v, cols):   # [T, 128*cols] -> [T, 128, cols] column-major chunks
        return np.ascontiguousarray(
            np.asarray(v, f32).reshape(T, cols, 128).transpose(0, 2, 1))

    bf = ml_dtypes.bfloat16
    xT_full = np.ascontiguousarray(inputs.transpose(0, 2, 1))      # [T, E, B]
    w1t = np.ascontiguousarray(W1.transpose(0, 2, 1))              # [T, E, H]
    w2t = np.ascontiguousarray(W2.transpose(0, 2, 1)).astype(bf)   # [T, H, V]
    wiht = np.ascontiguousarray(Wih.transpose(0, 2, 1)).astype(bf)
    whht = np.ascontiguousarray(Whh.transpose(0, 2, 1)).astype(bf)
    wrt = np.ascontiguousarray(Wr.transpose(0, 2, 1)).astype(bf)   # [T, H, 70]
    wot = np.ascontiguousarray(Wo.transpose(0, 2, 1)).astype(bf)   # [T, 576, E]
    h0t_full = np.asarray(h0, f32).transpose(0, 2, 1).astype(bf)
    c0t_full = np.ascontiguousarray(np.asarray(c0, f32).transpose(0, 2, 1))
    r0t_full = np.asarray(read0, f32)[T - 1].T.astype(bf)          # [M, B]
    wr0_full = np.asarray(wr0, f32)
    mem0_full = np.asarray(mem0).astype(ml_dtypes.bfloat16)
    bz = np.asarray(bih, f32) + np.asarray(bhh, f32)

    common = dict(
        w1t=w1t, w2t=w2t, wiht=wiht, whht=whht, wrt=wrt, wot=wot,
        b1c=percol(b1, HC), lngc=percol(lng, HC), lnbc=percol(lnb, HC),
        b2c=percol(b2, VC), bzc=percol(bz, ZC), bzch=percol(0.5 * bz, ZC),
        brc=np.ascontiguousarray(np.asarray(br, f32).reshape(T, M + 6, 1)),
        boc=percol(bo, EC),
    )
    in_maps = []
    for ci in range(NCORES):
        bsl = slice(ci * BS, (ci + 1) * BS)
        in_maps.append(dict(
            common,
            xT=np.ascontiguousarray(xT_full[:, :, bsl]),
            h0t=np.ascontiguousarray(h0t_full[:, :, bsl]),
            c0t=np.ascontiguousarray(c0t_full[:, :, bsl]),
            r0t=np.ascontiguousarray(r0t_full[:, bsl]),
            wr0=np.ascontiguousarray(wr0_full[:, bsl, :]),
            mem0=np.ascontiguousarray(mem0_full[:, bsl]),
        ))

    return in_maps


def kernel(**inputs):
    in_maps = host_prep(**inputs)
    nc = _get_nc()
    import os
    trace = os.environ.get("BASS_TRACE", "") not in ("", "0")
    res = run_bass_kernel_spmd(nc, in_maps, list(range(NCORES)), trace=trace)
    LAST["exec_time_ns"] = res.exec_time_ns
    LAST["results"] = res
    out = np.concatenate(
        [np.transpose(r["outT"], (0, 2, 1)) for r in res.results], axis=1)
    return np.ascontiguousarray(out.astype(np.float32))
